# revision 5
# baseline (speedup 1.0000x reference)
"""Trainium2 Bass kernel for nn_NeuralGraphHidden (GNN message passing).

Key insight: edges ~ randint(-1, 128) gives P(edge == -1) = 1/129, so ~95.5%
of atoms have degree 6 — and the reference's degree mask only covers degrees
0..5, so those atoms' outputs are EXACTLY ZERO.  Only atoms with degree < 6
("active" atoms, ~190 per core) ever contribute to the output, so the message
pipeline only needs their ~1150 edge slots, not all 196k.

The host shards the batch over 8 cores, buckets active atoms by degree
(uniform bucket capacities across cores so a single SPMD program serves all
8), and stages everything pre-transposed (feature-major) so the device never
transposes.  Neighbour atom features are staged per edge slot (cheap at this
sparsity), so the device pipeline is pure matmul + elementwise:

  pre    = W0a.T @ nbrT  +  W0b.T @ bondsT      (PSUM accumulate, per d-block)
  msg0   = elu(pre)      elu(x) = min(exp(x),1) + relu(x) - 1  (ACT exp + DVE)
  msg1   = elu(W1.T @ msg0)
  summed = sum_d msg1                            (d-major blocks, DVE adds)
  h0     = elu(W0d_hi.T @ summed + W0d_lo.T @ actT)    per degree bucket
  out    = elu(h0_chunk.T @ W1d)                 (data-stationary -> atom-major)

The host scatters the few computed rows into the (mostly zero) full output.
All math is f32; results match the reference to ~1e-5 relative error.
"""

import sys

if "/opt/trn_rl_repo" not in sys.path:
    sys.path.insert(0, "/opt/trn_rl_repo")

import numpy as np

import concourse.bass as bass
import concourse.bacc as bacc
import concourse.mybir as mybir
import concourse.tile as tile
from concourse import bass_utils

F32 = mybir.dt.float32
AF = mybir.ActivationFunctionType
ALU = mybir.AluOpType

B, M, D = 256, 128, 6
FA, FB, MSG, CONV = 128, 32, 128, 128
NCORES = 8
NMOL = B // NCORES           # molecules per core
NATOM = NMOL * M             # atoms per core (flat)


def _roundup(x, m):
    return (x + m - 1) // m * m


# --------------------------------------------------------------------------
# device program
# --------------------------------------------------------------------------

def build_program(NA, caps):
    """SPMD Bass program. NA: active-atom grid size; caps: per-degree bucket
    sizes (sum == NA), uniform across all 8 cores."""
    assert sum(caps) == NA

    nc = bacc.Bacc("TRN2", target_bir_lowering=False, debug=False,
                   num_devices=NCORES)

    def din(name, shape):
        return nc.dram_tensor(name, list(shape), F32, kind="ExternalInput").ap()

    nbrT_d = din("nbrT", (128, 6, NA))
    bondsT_d = din("bondsT", (32, 6, NA))
    actT_d = din("actT", (128, NA))
    w0a_d = din("w0a", (128, 128))
    w0b_d = din("w0b", (32, 128))
    w1_d = din("w1", (128, 128))
    iw0hi_d = din("iw0hi", (128, 6, 128))
    iw0lo_d = din("iw0lo", (128, 6, 128))
    iw1_d = din("iw1", (128, 6, 128))

    outp = nc.dram_tensor("outp", [NA, 128], F32, kind="ExternalOutput")
    outp_ap = outp.ap()

    with tile.TileContext(nc) as tc:
        with (
            tc.tile_pool(name="w", bufs=1) as wp,
            tc.tile_pool(name="big", bufs=1) as bigp,
            tc.tile_pool(name="work", bufs=2) as work,
            tc.tile_pool(name="psA", bufs=2, space=bass.MemorySpace.PSUM) as psA,
            tc.tile_pool(name="psM", bufs=2, space=bass.MemorySpace.PSUM) as psM,
        ):
            nbrT = wp.tile([128, 6, NA], F32, tag="nbrT")
            bondsT = wp.tile([32, 6, NA], F32, tag="bondsT")
            actT = wp.tile([128, NA], F32, tag="actT")
            w0a = wp.tile([128, 128], F32, tag="w0a")
            w0b = wp.tile([32, 128], F32, tag="w0b")
            w1 = wp.tile([128, 128], F32, tag="w1")
            iw0hi = wp.tile([128, 6, 128], F32, tag="iw0hi")
            iw0lo = wp.tile([128, 6, 128], F32, tag="iw0lo")
            iw1 = wp.tile([128, 6, 128], F32, tag="iw1")

            for t, d in [(nbrT, nbrT_d), (bondsT, bondsT_d), (actT, actT_d),
                         (w0a, w0a_d), (w0b, w0b_d), (w1, w1_d),
                         (iw0hi, iw0hi_d), (iw0lo, iw0lo_d), (iw1, iw1_d)]:
                nc.sync.dma_start(t[:], d[:])

            # ---- message MLP: 2 halves x 3 degree-blocks ----
            # psum tiles are [128, 3, 512] so each d-block owns a full bank
            # (independent accumulation groups); ops use the strided
            # [:, :, 0:NA] view so each layer is a single instruction.
            m1 = bigp.tile([128, 6, NA], F32, tag="m1")
            for h in range(2):
                pm = psM.tile([128, 3, 512], F32, tag="pm")
                pv = pm[:, :, 0:NA]
                for i in range(3):
                    nc.tensor.matmul(pm[:, i, 0:NA], w0a[:],
                                     nbrT[:, 3 * h + i, :],
                                     start=True, stop=False)
                for i in range(3):
                    nc.tensor.matmul(pm[:, i, 0:NA], w0b[:],
                                     bondsT[:, 3 * h + i, :],
                                     start=False, stop=True)
                e0 = work.tile([128, 3, NA], F32, tag="e0")
                r0 = work.tile([128, 3, NA], F32, tag="r0")
                m0 = work.tile([128, 3, NA], F32, tag="m0")
                nc.scalar.activation(e0[:], pv, AF.Exp)
                nc.vector.tensor_scalar(r0[:], pv, 0.0, -1.0,
                                        op0=ALU.max, op1=ALU.add)
                nc.vector.scalar_tensor_tensor(m0[:], e0[:], 1.0, r0[:],
                                               op0=ALU.min, op1=ALU.add)
                pm2 = psM.tile([128, 3, 512], F32, tag="pm")
                pv2 = pm2[:, :, 0:NA]
                for i in range(3):
                    nc.tensor.matmul(pm2[:, i, 0:NA], w1[:], m0[:, i, :],
                                     start=True, stop=True)
                e1 = work.tile([128, 3, NA], F32, tag="e0")
                r1 = work.tile([128, 3, NA], F32, tag="r0")
                nc.scalar.activation(e1[:], pv2, AF.Exp)
                nc.vector.tensor_scalar(r1[:], pv2, 0.0, -1.0,
                                        op0=ALU.max, op1=ALU.add)
                nc.vector.scalar_tensor_tensor(m1[:, 3 * h:3 * h + 3, :],
                                               e1[:], 1.0, r1[:],
                                               op0=ALU.min, op1=ALU.add)

            # ---- d-sum: summed = sum_d m1[:, d, :] ----
            summed = bigp.tile([128, NA], F32, tag="summed")
            stmp = work.tile([128, NA], F32, tag="stmp")
            nc.vector.tensor_tensor(summed[:], m1[:, 0, :], m1[:, 1, :], ALU.add)
            nc.vector.tensor_tensor(stmp[:], m1[:, 2, :], m1[:, 3, :], ALU.add)
            nc.vector.tensor_tensor(summed[:], summed[:], m1[:, 4, :], ALU.add)
            nc.vector.tensor_tensor(stmp[:], stmp[:], m1[:, 5, :], ALU.add)
            nc.vector.tensor_tensor(summed[:], summed[:], stmp[:], ALU.add)

            # ---- per-degree inner MLP, layer 0 ----
            h0 = bigp.tile([128, NA], F32, tag="h0")
            off = 0
            for d in range(D):
                cap = caps[d]
                if cap == 0:
                    continue
                for s0 in range(0, cap, 512):
                    w = min(512, cap - s0)
                    pi = psA.tile([128, 512], F32, tag="psA")
                    nc.tensor.matmul(pi[:, 0:w], iw0hi[:, d, :],
                                     summed[:, off + s0:off + s0 + w],
                                     start=True, stop=False)
                    nc.tensor.matmul(pi[:, 0:w], iw0lo[:, d, :],
                                     actT[:, off + s0:off + s0 + w],
                                     start=False, stop=True)
                    eh = work.tile([128, 512], F32, tag="eh")
                    rh = work.tile([128, 512], F32, tag="rh")
                    nc.scalar.activation(eh[:, 0:w], pi[:, 0:w], AF.Exp)
                    nc.vector.tensor_scalar(rh[:, 0:w], pi[:, 0:w], 0.0, -1.0,
                                            op0=ALU.max, op1=ALU.add)
                    nc.vector.scalar_tensor_tensor(
                        h0[:, off + s0:off + s0 + w], eh[:, 0:w], 1.0,
                        rh[:, 0:w], op0=ALU.min, op1=ALU.add)
                off += cap

            # ---- inner layer 1: data-stationary -> atom-major output ----
            off = 0
            for d in range(D):
                cap = caps[d]
                if cap == 0:
                    continue
                for s0 in range(0, cap, 128):
                    w = min(128, cap - s0)
                    po = psA.tile([128, 512], F32, tag="psA")
                    pov = po[0:w, 0:128]
                    nc.tensor.matmul(pov, h0[:, off + s0:off + s0 + w],
                                     iw1[:, d, :], start=True, stop=True)
                    eo = work.tile([128, 128], F32, tag="eo")
                    ro = work.tile([128, 128], F32, tag="ro")
                    ob = work.tile([128, 128], F32, tag="ob")
                    nc.scalar.activation(eo[0:w, :], pov, AF.Exp)
                    nc.vector.tensor_scalar(ro[0:w, :], pov, 0.0, -1.0,
                                            op0=ALU.max, op1=ALU.add)
                    nc.vector.scalar_tensor_tensor(ob[0:w, :], eo[0:w, :], 1.0,
                                                   ro[0:w, :],
                                                   op0=ALU.min, op1=ALU.add)
                    nc.sync.dma_start(outp_ap[off + s0:off + s0 + w, :],
                                      ob[0:w, :])
                off += cap

    nc.compile()
    return nc


_CACHE = {}


# --------------------------------------------------------------------------
# host side
# --------------------------------------------------------------------------

def _prep_core(atoms_c, bonds_c, edges_c, NA, caps):
    """Stage one core's arrays. Returns (dict name -> array, scatter info)."""
    af = atoms_c.reshape(NATOM, FA)
    bf = bonds_c.reshape(NATOM, D, FB)
    ef = edges_c.reshape(NATOM, D)
    deg = (ef != -1).sum(-1)

    act = np.nonzero(deg < D)[0]
    act = act[np.argsort(deg[act], kind="stable")]
    counts = np.bincount(deg[act], minlength=D)[:D]
    assert (counts <= np.asarray(caps)).all()

    # grid: bucket d occupies [S_d, S_d + caps[d]); real atoms first
    S = np.concatenate([[0], np.cumsum(caps)])[:D]
    grid = np.full(NA, -1, np.int64)
    ofs = S.copy()
    for a in act:
        d = deg[a]
        grid[ofs[d]] = a
        ofs[d] += 1

    real = grid >= 0
    ga = grid[real]

    actT = np.zeros((128, NA), np.float32)
    actT[:, real] = af[ga].T
    bondsT = np.zeros((32, D, NA), np.float32)
    bondsT[:, :, real] = bf[ga].transpose(2, 1, 0)

    # neighbour atom features per slot (d, grid pos)
    nbrT = np.zeros((128, D, NA), np.float32)
    e = ef[ga]                                   # (nreal, D) local indices
    mol = ga // M
    for d in range(D):
        has = e[:, d] >= 0
        cols = np.nonzero(real)[0][has]
        nbrT[:, d, cols] = af[mol[has] * M + e[has, d]].T

    return dict(nbrT=nbrT, actT=actT, bondsT=bondsT), ga, real


def _host_prep(atoms, bonds, edges):
    deg = (edges != -1).sum(-1).reshape(NCORES, NATOM)
    max_counts = np.zeros(D, np.int64)
    for c in range(NCORES):
        dc = deg[c]
        a = np.nonzero(dc < D)[0]
        cnt = np.bincount(dc[a], minlength=D)[:D]
        max_counts = np.maximum(max_counts, cnt)
    caps = [int(_roundup(x, 8)) if x > 0 else 0 for x in max_counts]
    NA = int(_roundup(max(sum(caps), 64), 64))
    caps[int(np.argmax(caps))] += NA - sum(caps)
    return NA, caps


def kernel(atoms, bonds, edges, msg_w0, msg_w1, inner_w0, inner_w1):
    atoms = np.asarray(atoms, np.float32)
    bonds = np.asarray(bonds, np.float32)
    edges = np.asarray(edges, np.int32)
    msg_w0 = np.asarray(msg_w0, np.float32)
    msg_w1 = np.asarray(msg_w1, np.float32)
    inner_w0 = np.asarray(inner_w0, np.float32)
    inner_w1 = np.asarray(inner_w1, np.float32)

    NA, caps = _host_prep(atoms, bonds, edges)

    key = (NA, tuple(caps))
    if key not in _CACHE:
        _CACHE[key] = build_program(NA, caps)
    nc = _CACHE[key]

    shared = dict(
        w0a=msg_w0[:128].copy(),
        w0b=msg_w0[128:160].copy(),
        w1=msg_w1,
        iw0hi=np.ascontiguousarray(inner_w0[:, :128, :].transpose(1, 0, 2)),
        iw0lo=np.ascontiguousarray(inner_w0[:, 128:, :].transpose(1, 0, 2)),
        iw1=np.ascontiguousarray(inner_w1.transpose(1, 0, 2)),
    )

    in_maps = []
    scatter = []
    for c in range(NCORES):
        sl = slice(c * NMOL, (c + 1) * NMOL)
        m, ga, real = _prep_core(atoms[sl], bonds[sl], edges[sl], NA, caps)
        m.update(shared)
        in_maps.append(m)
        scatter.append((ga, real))

    res = bass_utils.run_bass_kernel_spmd(
        nc, in_maps, core_ids=list(range(NCORES)))

    out = np.zeros((B * M, CONV), np.float32)
    for c in range(NCORES):
        ga, real = scatter[c]
        o = res.results[c]["outp"]
        out[c * NATOM + ga] = o[real]
    return out.reshape(B, M, CONV)


# revision 7
# speedup vs baseline: 1.4141x; 1.4141x over previous
"""Trainium2 Bass kernel for nn_NeuralGraphHidden (GNN message passing).

Key insight: edges ~ randint(-1, 128) gives P(edge == -1) = 1/129, so ~95.5%
of atoms have degree 6 — and the reference's degree mask only covers degrees
0..5, so those atoms' outputs are EXACTLY ZERO.  Only atoms with degree < 6
("active" atoms, ~190 per core) ever contribute to the output, so the message
pipeline only needs their ~1150 edge slots, not all 196k.

The host shards the batch over 8 cores, buckets active atoms by degree
(uniform bucket capacities across cores so a single SPMD program serves all
8), and stages everything pre-transposed (feature-major) so the device never
transposes.  Neighbour atom features are staged per edge slot (cheap at this
sparsity), so the device pipeline is pure matmul + elementwise:

  pre    = W0a.T @ nbrT  +  W0b.T @ bondsT      (PSUM accumulate, per d-block)
  msg0   = elu(pre)      elu(x) = min(exp(x),1) + relu(x) - 1  (ACT exp + DVE)
  msg1   = elu(W1.T @ msg0)
  summed = sum_d msg1                            (d-major blocks, DVE adds)
  h0     = elu(W0d_hi.T @ summed + W0d_lo.T @ actT)    per degree bucket
  out    = elu(h0_chunk.T @ W1d)                 (data-stationary -> atom-major)

Matmul operands are bf16 (PE streams 4x faster than f32 and FWL halves
LDWEIGHTS); accumulation and all elu math stay f32 via PSUM.  The host
scatters the few computed rows into the (mostly zero) full output.
"""

import sys

if "/opt/trn_rl_repo" not in sys.path:
    sys.path.insert(0, "/opt/trn_rl_repo")

import numpy as np
import ml_dtypes

import concourse.bass as bass
import concourse.bacc as bacc
import concourse.mybir as mybir
import concourse.tile as tile
from concourse import bass_utils

BF16 = ml_dtypes.bfloat16
F32 = mybir.dt.float32
F32R = mybir.dt.float32r
BF = mybir.dt.bfloat16
AF = mybir.ActivationFunctionType
ALU = mybir.AluOpType

B, M, D = 256, 128, 6
FA, FB, MSG, CONV = 128, 32, 128, 128
NCORES = 8
NMOL = B // NCORES           # molecules per core
NATOM = NMOL * M             # atoms per core (flat)


def _roundup(x, m):
    return (x + m - 1) // m * m


# --------------------------------------------------------------------------
# device program
# --------------------------------------------------------------------------

def build_program(NA, caps):
    """SPMD Bass program. NA: active-atom grid size; caps: per-degree bucket
    sizes (sum == NA), uniform across all 8 cores."""
    assert sum(caps) == NA

    nc = bacc.Bacc("TRN2", target_bir_lowering=False, debug=False,
                   num_devices=NCORES)

    # packed inputs: one DMA each
    #   na:    [128, 7, NA]  bf16 -- nbrT (6 d-blocks) + actT
    #   bo:    [32, 6, NA]   bf16 -- bondsT
    #   wpk:   [128, 21, 128] bf16 -- w0a | w0b(pad) | w1 | iw0hi*6 | iw0lo*6 | iw1*6
    na_d = nc.dram_tensor("na", [128, 7, NA], F32R, kind="ExternalInput").ap()
    bo_d = nc.dram_tensor("bo", [32, 6, NA], F32R, kind="ExternalInput").ap()
    wpk_d = nc.dram_tensor("wpk", [128, 21, 128], F32R, kind="ExternalInput").ap()

    outp = nc.dram_tensor("outp", [NA, 128], F32, kind="ExternalOutput")
    outp_ap = outp.ap()

    with tile.TileContext(nc) as tc:
        with (
            tc.tile_pool(name="w", bufs=1) as wp,
            tc.tile_pool(name="big", bufs=1) as bigp,
            tc.tile_pool(name="work", bufs=4) as work,
            tc.tile_pool(name="psA", bufs=2, space=bass.MemorySpace.PSUM) as psA,
            tc.tile_pool(name="psM", bufs=2, space=bass.MemorySpace.PSUM) as psM,
        ):
            na = wp.tile([128, 7, NA], F32R, tag="na")
            bo = wp.tile([32, 6, NA], F32R, tag="bo")
            wpk = wp.tile([128, 21, 128], F32R, tag="wpk")
            nc.sync.dma_start(na[:], na_d[:])
            nc.sync.dma_start(bo[:], bo_d[:])
            nc.sync.dma_start(wpk[:], wpk_d[:])

            def nbrT(d):
                return na[:, d, :]
            actT = na[:, 6, :]
            w0a = wpk[:, 0, :]
            w0b = wpk[0:32, 1, :]
            w1 = wpk[:, 2, :]

            def iw0hi(d):
                return wpk[:, 3 + d, :]

            def iw0lo(d):
                return wpk[:, 9 + d, :]

            def iw1(d):
                return wpk[:, 15 + d, :]

            # ---- message MLP: 2 halves x 3 degree-blocks ----
            # psum tiles are [128, 3, 512] so each d-block owns a full bank
            # (independent accumulation groups); elementwise ops use the
            # strided [:, :, 0:NA] view so each layer is one instruction.
            m1 = bigp.tile([128, 6, NA], F32R, tag="m1")
            for h in range(2):
                pm = psM.tile([128, 3, 512], F32, tag="pm")
                pv = pm[:, :, 0:NA]
                for i in range(3):
                    nc.tensor.matmul(pm[:, i, 0:NA], w0a, nbrT(3 * h + i),
                                     start=True, stop=False)
                for i in range(3):
                    nc.tensor.matmul(pm[:, i, 0:NA], w0b, bo[:, 3 * h + i, :],
                                     start=False, stop=True)
                e0 = work.tile([128, 3, NA], F32R, tag="e0")
                r0 = work.tile([128, 3, NA], F32R, tag="r0")
                m0 = work.tile([128, 3, NA], F32R, tag="m0")
                nc.scalar.activation(e0[:], pv, AF.Exp)
                nc.vector.tensor_scalar(r0[:], pv, 0.0, -1.0,
                                        op0=ALU.max, op1=ALU.add)
                nc.vector.scalar_tensor_tensor(m0[:], e0[:], 1.0, r0[:],
                                               op0=ALU.min, op1=ALU.add)
                pm2 = psM.tile([128, 3, 512], F32, tag="pm")
                pv2 = pm2[:, :, 0:NA]
                for i in range(3):
                    nc.tensor.matmul(pm2[:, i, 0:NA], w1, m0[:, i, :],
                                     start=True, stop=True)
                e1 = work.tile([128, 3, NA], F32R, tag="e0")
                r1 = work.tile([128, 3, NA], F32R, tag="r0")
                nc.scalar.activation(e1[:], pv2, AF.Exp)
                nc.vector.tensor_scalar(r1[:], pv2, 0.0, -1.0,
                                        op0=ALU.max, op1=ALU.add)
                nc.vector.scalar_tensor_tensor(m1[:, 3 * h:3 * h + 3, :],
                                               e1[:], 1.0, r1[:],
                                               op0=ALU.min, op1=ALU.add)

            # ---- d-sum: summed = sum_d m1[:, d, :]  (bf16, 2x mode) ----
            summed = bigp.tile([128, NA], F32R, tag="summed")
            stmp = work.tile([128, NA], F32R, tag="stmp")
            nc.vector.tensor_tensor(summed[:], m1[:, 0, :], m1[:, 1, :], ALU.add)
            nc.vector.tensor_tensor(stmp[:], m1[:, 2, :], m1[:, 3, :], ALU.add)
            nc.vector.tensor_tensor(summed[:], summed[:], m1[:, 4, :], ALU.add)
            nc.vector.tensor_tensor(stmp[:], stmp[:], m1[:, 5, :], ALU.add)
            nc.vector.tensor_tensor(summed[:], summed[:], stmp[:], ALU.add)

            # ---- per-degree inner MLP, layer 0 ----
            h0 = bigp.tile([128, NA], F32R, tag="h0")
            off = 0
            for d in range(D):
                cap = caps[d]
                if cap == 0:
                    continue
                for s0 in range(0, cap, 512):
                    w = min(512, cap - s0)
                    pi = psA.tile([128, 512], F32, tag="psA")
                    nc.tensor.matmul(pi[:, 0:w], iw0hi(d),
                                     summed[:, off + s0:off + s0 + w],
                                     start=True, stop=False)
                    nc.tensor.matmul(pi[:, 0:w], iw0lo(d),
                                     actT[:, off + s0:off + s0 + w],
                                     start=False, stop=True)
                    eh = work.tile([128, 512], F32R, tag="eh")
                    rh = work.tile([128, 512], F32R, tag="rh")
                    nc.scalar.activation(eh[:, 0:w], pi[:, 0:w], AF.Exp)
                    nc.vector.tensor_scalar(rh[:, 0:w], pi[:, 0:w], 0.0, -1.0,
                                            op0=ALU.max, op1=ALU.add)
                    nc.vector.scalar_tensor_tensor(
                        h0[:, off + s0:off + s0 + w], eh[:, 0:w], 1.0,
                        rh[:, 0:w], op0=ALU.min, op1=ALU.add)
                off += cap

            # ---- inner layer 1: data-stationary -> atom-major output ----
            off = 0
            for d in range(D):
                cap = caps[d]
                if cap == 0:
                    continue
                for s0 in range(0, cap, 128):
                    w = min(128, cap - s0)
                    po = psA.tile([128, 512], F32, tag="psA")
                    pov = po[0:w, 0:128]
                    nc.tensor.matmul(pov, h0[:, off + s0:off + s0 + w],
                                     iw1(d), start=True, stop=True)
                    eo = work.tile([128, 128], F32, tag="eo")
                    ro = work.tile([128, 128], F32, tag="ro")
                    ob = work.tile([128, 128], F32, tag="ob")
                    nc.scalar.activation(eo[0:w, :], pov, AF.Exp)
                    nc.vector.tensor_scalar(ro[0:w, :], pov, 0.0, -1.0,
                                            op0=ALU.max, op1=ALU.add)
                    nc.vector.scalar_tensor_tensor(ob[0:w, :], eo[0:w, :], 1.0,
                                                   ro[0:w, :],
                                                   op0=ALU.min, op1=ALU.add)
                    nc.sync.dma_start(outp_ap[off + s0:off + s0 + w, :],
                                      ob[0:w, :])
                off += cap

    nc.compile()
    return nc


_CACHE = {}


# --------------------------------------------------------------------------
# host side
# --------------------------------------------------------------------------

def _prep_core(atoms_c, bonds_c, edges_c, NA, caps):
    """Stage one core's arrays. Returns (dict name -> array, scatter info)."""
    af = atoms_c.reshape(NATOM, FA)
    bf = bonds_c.reshape(NATOM, D, FB)
    ef = edges_c.reshape(NATOM, D)
    deg = (ef != -1).sum(-1)

    act = np.nonzero(deg < D)[0]
    act = act[np.argsort(deg[act], kind="stable")]
    counts = np.bincount(deg[act], minlength=D)[:D]
    assert (counts <= np.asarray(caps)).all()

    # grid: bucket d occupies [S_d, S_d + caps[d]); real atoms first
    S = np.concatenate([[0], np.cumsum(caps)])[:D]
    grid = np.full(NA, -1, np.int64)
    ofs = S.copy()
    for a in act:
        d = deg[a]
        grid[ofs[d]] = a
        ofs[d] += 1

    real = grid >= 0
    ga = grid[real]

    na = np.zeros((128, 7, NA), np.float32)
    na[:, 6, real] = af[ga].T                     # actT
    bo = np.zeros((32, D, NA), np.float32)
    bo[:, :, real] = bf[ga].transpose(2, 1, 0)

    e = ef[ga]                                    # (nreal, D) local indices
    mol = ga // M
    rcols = np.nonzero(real)[0]
    for d in range(D):
        has = e[:, d] >= 0
        na[:, d, rcols[has]] = af[mol[has] * M + e[has, d]].T

    return dict(na=na, bo=bo), ga, real


def _host_prep(atoms, bonds, edges):
    deg = (edges != -1).sum(-1).reshape(NCORES, NATOM)
    max_counts = np.zeros(D, np.int64)
    for c in range(NCORES):
        dc = deg[c]
        a = np.nonzero(dc < D)[0]
        cnt = np.bincount(dc[a], minlength=D)[:D]
        max_counts = np.maximum(max_counts, cnt)
    caps = [int(_roundup(x, 8)) if x > 0 else 0 for x in max_counts]
    NA = int(_roundup(max(sum(caps), 256), 64))
    caps[int(np.argmax(caps))] += NA - sum(caps)
    return NA, caps


def _pack_weights(msg_w0, msg_w1, inner_w0, inner_w1):
    wpk = np.zeros((128, 21, 128), np.float32)
    wpk[:, 0, :] = msg_w0[:128]
    wpk[0:32, 1, :] = msg_w0[128:160]
    wpk[:, 2, :] = msg_w1
    wpk[:, 3:9, :] = inner_w0[:, :128, :].transpose(1, 0, 2)
    wpk[:, 9:15, :] = inner_w0[:, 128:, :].transpose(1, 0, 2)
    wpk[:, 15:21, :] = inner_w1.transpose(1, 0, 2)
    return wpk


def kernel(atoms, bonds, edges, msg_w0, msg_w1, inner_w0, inner_w1):
    atoms = np.asarray(atoms, np.float32)
    bonds = np.asarray(bonds, np.float32)
    edges = np.asarray(edges, np.int32)
    msg_w0 = np.asarray(msg_w0, np.float32)
    msg_w1 = np.asarray(msg_w1, np.float32)
    inner_w0 = np.asarray(inner_w0, np.float32)
    inner_w1 = np.asarray(inner_w1, np.float32)

    NA, caps = _host_prep(atoms, bonds, edges)

    key = (NA, tuple(caps))
    if key not in _CACHE:
        _CACHE[key] = build_program(NA, caps)
    nc = _CACHE[key]

    wpk = _pack_weights(msg_w0, msg_w1, inner_w0, inner_w1)

    in_maps = []
    scatter = []
    for c in range(NCORES):
        sl = slice(c * NMOL, (c + 1) * NMOL)
        m, ga, real = _prep_core(atoms[sl], bonds[sl], edges[sl], NA, caps)
        m["wpk"] = wpk
        in_maps.append(m)
        scatter.append((ga, real))

    res = bass_utils.run_bass_kernel_spmd(
        nc, in_maps, core_ids=list(range(NCORES)))

    out = np.zeros((B * M, CONV), np.float32)
    for c in range(NCORES):
        ga, real = scatter[c]
        o = res.results[c]["outp"]
        out[c * NATOM + ga] = o[real]
    return out.reshape(B, M, CONV)


# revision 10
# speedup vs baseline: 1.4250x; 1.0077x over previous
"""Trainium2 Bass kernel for nn_NeuralGraphHidden (GNN message passing).

Key insight: edges ~ randint(-1, 128) gives P(edge == -1) = 1/129, so ~95.5%
of atoms have degree 6 — and the reference's degree mask only covers degrees
0..5, so those atoms' outputs are EXACTLY ZERO.  Only atoms with degree < 6
("active" atoms, ~190 per core) ever contribute to the output, so the message
pipeline only needs their ~1150 edge slots, not all 196k.

The host shards the batch over 8 cores, buckets active atoms by degree
(uniform bucket capacities across cores so a single SPMD program serves all
8), and stages everything pre-transposed (feature-major) so the device never
transposes.  Neighbour atom features are staged per edge slot (cheap at this
sparsity), so the device pipeline is pure matmul + elementwise, per degree
block d:

  pre_d  = W0a.T @ nbrT_d  +  W0b.T @ bondsT_d   (PSUM accumulate)
  msg0_d = elu(pre_d)    elu(x) = min(exp(x),1) + relu(x) - 1  (ACT exp + DVE)
  msg1_d = elu(W1.T @ msg0_d)
  summed = sum_d msg1_d                          (DVE adds, tree)
  h0     = elu(W0d_hi.T @ summed + W0d_lo.T @ actT)    per degree bucket
  out    = elu(h0_chunk.T @ W1d)                 (data-stationary -> atom-major)

Matmul operands are float32r (PE streams fp32 ~2-4x faster than plain
float32); accumulation and elu math stay f32 via PSUM.  Inputs are DMA'd in
dependency order so the first matmuls overlap the remaining loads, and a
short warm-up matmul burst during the DMA wait ramps the PE clock.
The host scatters the few computed rows into the (mostly zero) full output.
"""

import sys

if "/opt/trn_rl_repo" not in sys.path:
    sys.path.insert(0, "/opt/trn_rl_repo")

import numpy as np
import ml_dtypes

import concourse.bass as bass
import concourse.bacc as bacc
import concourse.mybir as mybir
import concourse.tile as tile
from concourse import bass_utils

BF16 = ml_dtypes.bfloat16
F32 = mybir.dt.float32
F32R = mybir.dt.float32r
BF = mybir.dt.bfloat16
AF = mybir.ActivationFunctionType
ALU = mybir.AluOpType

B, M, D = 256, 128, 6
FA, FB, MSG, CONV = 128, 32, 128, 128
NCORES = 8
NMOL = B // NCORES           # molecules per core
NATOM = NMOL * M             # atoms per core (flat)

WARMUP_MMS = 16              # PE clock-ramp burst during input DMA wait


def _roundup(x, m):
    return (x + m - 1) // m * m


def _chunks(caps):
    """h1 output chunks: (degree, start-within-bucket, width)."""
    out = []
    for d in range(D):
        cap = caps[d]
        for s0 in range(0, cap, 128):
            out.append((d, s0, min(128, cap - s0)))
    return out


# --------------------------------------------------------------------------
# device program
# --------------------------------------------------------------------------

def build_program(NA, caps, warmup=WARMUP_MMS):
    """SPMD Bass program. NA: active-atom grid size; caps: per-degree bucket
    sizes (sum == NA), uniform across all 8 cores."""
    assert sum(caps) == NA
    chunks = _chunks(caps)
    NCH = len(chunks)

    nc = bacc.Bacc("TRN2", target_bir_lowering=False, debug=False,
                   num_devices=NCORES)

    def din(name, shape):
        return nc.dram_tensor(name, list(shape), F32R,
                              kind="ExternalInput").ap()

    wmsg_d = din("wmsg", (128, 3, 128))     # w0a | w0b(pad) | w1
    na0_d = din("na0", (128, 3, NA))        # nbrT d=0..2
    bo0_d = din("bo0", (32, 3, NA))
    na1_d = din("na1", (128, 3, NA))        # nbrT d=3..5
    bo1_d = din("bo1", (32, 3, NA))
    nact_d = din("nact", (128, NA))         # actT
    winn_d = din("winn", (128, 18, 128))    # iw0hi*6 | iw0lo*6 | iw1*6

    outp = nc.dram_tensor("outp", [NCH * 128, 128], F32,
                          kind="ExternalOutput")
    outp_ap = outp.ap()

    with tile.TileContext(nc) as tc:
        with (
            tc.tile_pool(name="w", bufs=1) as wp,
            tc.tile_pool(name="big", bufs=1) as bigp,
            tc.tile_pool(name="work", bufs=6) as work,
            tc.tile_pool(name="psM", bufs=4, space=bass.MemorySpace.PSUM) as psM,
            tc.tile_pool(name="psA", bufs=2, space=bass.MemorySpace.PSUM) as psA,
            tc.tile_pool(name="psW", bufs=1, space=bass.MemorySpace.PSUM) as psW,
        ):
            wmsg = wp.tile([128, 3, 128], F32R, tag="wmsg")
            na0 = wp.tile([128, 3, NA], F32R, tag="na0")
            bo0 = wp.tile([32, 3, NA], F32R, tag="bo0")
            na1 = wp.tile([128, 3, NA], F32R, tag="na1")
            bo1 = wp.tile([32, 3, NA], F32R, tag="bo1")
            nact = wp.tile([128, NA], F32R, tag="nact")
            winn = wp.tile([128, 18, 128], F32R, tag="winn")
            nc.sync.dma_start(wmsg[:], wmsg_d[:])
            nc.sync.dma_start(na0[:], na0_d[:])
            nc.sync.dma_start(bo0[:], bo0_d[:])
            nc.sync.dma_start(na1[:], na1_d[:])
            nc.sync.dma_start(bo1[:], bo1_d[:])
            nc.sync.dma_start(nact[:], nact_d[:])
            nc.sync.dma_start(winn[:], winn_d[:])

            w0a = wmsg[:, 0, :]
            w0b = wmsg[0:32, 1, :]
            w1 = wmsg[:, 2, :]

            def nbrT(d):
                return (na0 if d < 3 else na1)[:, d % 3, :]

            def boT(d):
                return (bo0 if d < 3 else bo1)[:, d % 3, :]

            def iw0hi(d):
                return winn[:, d, :]

            def iw0lo(d):
                return winn[:, 6 + d, :]

            def iw1(d):
                return winn[:, 12 + d, :]

            # ---- PE clock-ramp burst (no data deps; runs during DMA wait) --
            if warmup:
                wz = wp.tile([128, 256], BF, tag="wz")
                nc.vector.memset(wz[:], 0.0)
                pw = psW.tile([128, 512], F32, tag="psW")
                for _ in range(warmup):
                    nc.tensor.matmul(pw[:, 0:256], wz[:, 0:128], wz[:, 0:256],
                                     start=True, stop=True)

            # ---- message MLP, one degree block at a time ----
            m1 = bigp.tile([128, 6, NA], F32R, tag="m1")
            for d in range(D):
                pm = psM.tile([128, 512], F32, tag="pm")
                pv = pm[:, 0:NA]
                nc.tensor.matmul(pv, w0a, nbrT(d), start=True, stop=False)
                nc.tensor.matmul(pv, w0b, boT(d), start=False, stop=True)
                e0 = work.tile([128, NA], F32R, tag="e0")
                r0 = work.tile([128, NA], F32R, tag="r0")
                m0 = work.tile([128, NA], F32R, tag="m0")
                nc.scalar.activation(e0[:], pv, AF.Exp)
                nc.vector.tensor_scalar(r0[:], pv, 0.0, -1.0,
                                        op0=ALU.max, op1=ALU.add)
                nc.vector.scalar_tensor_tensor(m0[:], e0[:], 1.0, r0[:],
                                               op0=ALU.min, op1=ALU.add)
                pm2 = psM.tile([128, 512], F32, tag="pm")
                pv2 = pm2[:, 0:NA]
                nc.tensor.matmul(pv2, w1, m0[:], start=True, stop=True)
                e1 = work.tile([128, NA], F32R, tag="e0")
                r1 = work.tile([128, NA], F32R, tag="r0")
                nc.scalar.activation(e1[:], pv2, AF.Exp)
                nc.vector.tensor_scalar(r1[:], pv2, 0.0, -1.0,
                                        op0=ALU.max, op1=ALU.add)
                nc.vector.scalar_tensor_tensor(m1[:, d, :], e1[:], 1.0, r1[:],
                                               op0=ALU.min, op1=ALU.add)

            # ---- d-sum tree (interleaves with later blocks' compute) ----
            s01 = work.tile([128, NA], F32R, tag="s01")
            s23 = work.tile([128, NA], F32R, tag="s23")
            summed = bigp.tile([128, NA], F32R, tag="summed")
            nc.vector.tensor_tensor(s01[:], m1[:, 0, :], m1[:, 1, :], ALU.add)
            nc.vector.tensor_tensor(s23[:], m1[:, 2, :], m1[:, 3, :], ALU.add)
            nc.vector.tensor_tensor(s01[:], s01[:], m1[:, 4, :], ALU.add)
            nc.vector.tensor_tensor(s23[:], s23[:], m1[:, 5, :], ALU.add)
            nc.vector.tensor_tensor(summed[:], s01[:], s23[:], ALU.add)

            # ---- per-degree inner MLP, layer 0 ----
            h0 = bigp.tile([128, NA], F32R, tag="h0")
            off = 0
            for d in range(D):
                cap = caps[d]
                if cap == 0:
                    continue
                for s0 in range(0, cap, 512):
                    w = min(512, cap - s0)
                    pi = psA.tile([128, 512], F32, tag="psA")
                    nc.tensor.matmul(pi[:, 0:w], iw0hi(d),
                                     summed[:, off + s0:off + s0 + w],
                                     start=True, stop=False)
                    nc.tensor.matmul(pi[:, 0:w], iw0lo(d),
                                     nact[:, off + s0:off + s0 + w],
                                     start=False, stop=True)
                    eh = work.tile([128, 512], F32R, tag="eh")
                    rh = work.tile([128, 512], F32R, tag="rh")
                    nc.scalar.activation(eh[:, 0:w], pi[:, 0:w], AF.Exp)
                    nc.vector.tensor_scalar(rh[:, 0:w], pi[:, 0:w], 0.0, -1.0,
                                            op0=ALU.max, op1=ALU.add)
                    nc.vector.scalar_tensor_tensor(
                        h0[:, off + s0:off + s0 + w], eh[:, 0:w], 1.0,
                        rh[:, 0:w], op0=ALU.min, op1=ALU.add)
                off += cap

            # ---- inner layer 1 -> single chunk-major output DMA ----
            obuf = bigp.tile([128, NCH, 128], F32, tag="obuf")
            nc.gpsimd.memset(obuf[:], 0.0)
            S = [0] * D
            acc = 0
            for d in range(D):
                S[d] = acc
                acc += caps[d]
            for k, (d, s0, w) in enumerate(chunks):
                po = psA.tile([128, 512], F32, tag="psA")
                pov = po[0:w, 0:128]
                col = S[d] + s0
                nc.tensor.matmul(pov, h0[:, col:col + w], iw1(d),
                                 start=True, stop=True)
                eo = work.tile([128, 128], F32, tag="eo")
                ro = work.tile([128, 128], F32, tag="ro")
                nc.scalar.activation(eo[0:w, :], pov, AF.Exp)
                nc.vector.tensor_scalar(ro[0:w, :], pov, 0.0, -1.0,
                                        op0=ALU.max, op1=ALU.add)
                nc.vector.scalar_tensor_tensor(obuf[0:w, k, :], eo[0:w, :],
                                               1.0, ro[0:w, :],
                                               op0=ALU.min, op1=ALU.add)
            nc.sync.dma_start(
                outp_ap.rearrange("(k p) c -> p k c", p=128), obuf[:])

    nc.compile()
    return nc


_CACHE = {}


# --------------------------------------------------------------------------
# host side
# --------------------------------------------------------------------------

def _prep_core(atoms_c, bonds_c, edges_c, NA, caps):
    """Stage one core's arrays. Returns (dict name -> array, scatter info)."""
    af = atoms_c.reshape(NATOM, FA)
    bf = bonds_c.reshape(NATOM, D, FB)
    ef = edges_c.reshape(NATOM, D)
    deg = (ef != -1).sum(-1)

    act = np.nonzero(deg < D)[0]
    act = act[np.argsort(deg[act], kind="stable")]
    counts = np.bincount(deg[act], minlength=D)[:D]
    assert (counts <= np.asarray(caps)).all()

    S = np.concatenate([[0], np.cumsum(caps)])[:D]
    grid = np.full(NA, -1, np.int64)
    ofs = S.copy()
    for a in act:
        d = deg[a]
        grid[ofs[d]] = a
        ofs[d] += 1

    real = grid >= 0
    ga = grid[real]

    nbrT = np.zeros((128, D, NA), np.float32)
    e = ef[ga]
    mol = ga // M
    rcols = np.nonzero(real)[0]
    for d in range(D):
        has = e[:, d] >= 0
        nbrT[:, d, rcols[has]] = af[mol[has] * M + e[has, d]].T

    bo = np.zeros((32, D, NA), np.float32)
    bo[:, :, real] = bf[ga].transpose(2, 1, 0)
    nact = np.zeros((128, NA), np.float32)
    nact[:, real] = af[ga].T

    m = dict(
        na0=np.ascontiguousarray(nbrT[:, 0:3, :]),
        na1=np.ascontiguousarray(nbrT[:, 3:6, :]),
        bo0=np.ascontiguousarray(bo[:, 0:3, :]),
        bo1=np.ascontiguousarray(bo[:, 3:6, :]),
        nact=nact,
    )
    return m, ga, real


def _host_prep(atoms, bonds, edges):
    deg = (edges != -1).sum(-1).reshape(NCORES, NATOM)
    max_counts = np.zeros(D, np.int64)
    for c in range(NCORES):
        dc = deg[c]
        a = np.nonzero(dc < D)[0]
        cnt = np.bincount(dc[a], minlength=D)[:D]
        max_counts = np.maximum(max_counts, cnt)
    caps = [int(_roundup(x, 8)) if x > 0 else 0 for x in max_counts]
    NA = int(_roundup(max(sum(caps), 256), 64))
    caps[int(np.argmax(caps))] += NA - sum(caps)
    return NA, caps


def _pack_weights(msg_w0, msg_w1, inner_w0, inner_w1):
    wmsg = np.zeros((128, 3, 128), np.float32)
    wmsg[:, 0, :] = msg_w0[:128]
    wmsg[0:32, 1, :] = msg_w0[128:160]
    wmsg[:, 2, :] = msg_w1
    winn = np.zeros((128, 18, 128), np.float32)
    winn[:, 0:6, :] = inner_w0[:, :128, :].transpose(1, 0, 2)
    winn[:, 6:12, :] = inner_w0[:, 128:, :].transpose(1, 0, 2)
    winn[:, 12:18, :] = inner_w1.transpose(1, 0, 2)
    return wmsg, winn


def kernel(atoms, bonds, edges, msg_w0, msg_w1, inner_w0, inner_w1):
    atoms = np.asarray(atoms, np.float32)
    bonds = np.asarray(bonds, np.float32)
    edges = np.asarray(edges, np.int32)
    msg_w0 = np.asarray(msg_w0, np.float32)
    msg_w1 = np.asarray(msg_w1, np.float32)
    inner_w0 = np.asarray(inner_w0, np.float32)
    inner_w1 = np.asarray(inner_w1, np.float32)

    NA, caps = _host_prep(atoms, bonds, edges)

    key = (NA, tuple(caps))
    if key not in _CACHE:
        _CACHE[key] = build_program(NA, caps)
    nc = _CACHE[key]

    wmsg, winn = _pack_weights(msg_w0, msg_w1, inner_w0, inner_w1)

    in_maps = []
    scatter = []
    for c in range(NCORES):
        sl = slice(c * NMOL, (c + 1) * NMOL)
        m, ga, real = _prep_core(atoms[sl], bonds[sl], edges[sl], NA, caps)
        m["wmsg"] = wmsg
        m["winn"] = winn
        in_maps.append(m)
        scatter.append((ga, real))

    res = bass_utils.run_bass_kernel_spmd(
        nc, in_maps, core_ids=list(range(NCORES)))

    # unscatter: output rows are chunk-major (d, s0, w)
    chunks = _chunks(caps)
    S = np.concatenate([[0], np.cumsum(caps)])[:D]
    out = np.zeros((B * M, CONV), np.float32)
    for c in range(NCORES):
        ga, real = scatter[c]
        o = res.results[c]["outp"]
        full = np.zeros((NA, CONV), np.float32)
        for k, (d, s0, w) in enumerate(chunks):
            full[S[d] + s0:S[d] + s0 + w] = o[k * 128:k * 128 + w]
        out[c * NATOM + ga] = full[real]
    return out.reshape(B, M, CONV)


# revision 12
# speedup vs baseline: 1.4731x; 1.0337x over previous
"""Trainium2 Bass kernel for nn_NeuralGraphHidden (GNN message passing).

Key insight: edges ~ randint(-1, 128) gives P(edge == -1) = 1/129, so ~95.5%
of atoms have degree 6 — and the reference's degree mask only covers degrees
0..5, so those atoms' outputs are EXACTLY ZERO.  Only atoms with degree < 6
("active" atoms, ~190 per core) ever contribute to the output, so the message
pipeline only needs their ~1150 edge slots, not all 196k.

The host shards the batch over 8 cores, buckets active atoms by degree
(uniform bucket capacities across cores so a single SPMD program serves all
8), and stages everything pre-transposed (feature-major) so the device never
transposes.  Neighbour atom features are staged per edge slot (cheap at this
sparsity), so the device pipeline is pure matmul + elementwise, per degree
block d:

  pre_d  = W0a.T @ nbrT_d  +  W0b.T @ bondsT_d   (PSUM accumulate)
  msg0_d = elu(pre_d)    elu(x) = min(exp(x),1) + relu(x) - 1  (ACT exp + DVE)
  msg1_d = elu(W1.T @ msg0_d)
  summed = sum_d msg1_d                          (DVE adds, tree)
  h0     = elu(W0d_hi.T @ summed + W0d_lo.T @ actT)    per degree bucket
  out    = elu(h0_chunk.T @ W1d)                 (data-stationary -> atom-major)

Matmul operands are float32r (PE streams fp32 ~2-4x faster than plain
float32); accumulation and elu math stay f32 via PSUM.  Inputs are DMA'd in
dependency order so the first matmuls overlap the remaining loads, and a
short warm-up matmul burst during the DMA wait ramps the PE clock.
The host scatters the few computed rows into the (mostly zero) full output.
"""

import sys

if "/opt/trn_rl_repo" not in sys.path:
    sys.path.insert(0, "/opt/trn_rl_repo")

import numpy as np
import ml_dtypes

import concourse.bass as bass
import concourse.bacc as bacc
import concourse.mybir as mybir
import concourse.tile as tile
from concourse import bass_utils

BF16 = ml_dtypes.bfloat16
F32 = mybir.dt.float32
F32R = mybir.dt.float32r
BF = mybir.dt.bfloat16
AF = mybir.ActivationFunctionType
ALU = mybir.AluOpType

B, M, D = 256, 128, 6
FA, FB, MSG, CONV = 128, 32, 128, 128
NCORES = 8
NMOL = B // NCORES           # molecules per core
NATOM = NMOL * M             # atoms per core (flat)

WARMUP_MMS = 0               # PE clock-ramp burst (measured: no effect)


def _roundup(x, m):
    return (x + m - 1) // m * m


def _chunks(caps):
    """h1 output chunks: (degree, start-within-bucket, width)."""
    out = []
    for d in range(D):
        cap = caps[d]
        for s0 in range(0, cap, 128):
            out.append((d, s0, min(128, cap - s0)))
    return out


# --------------------------------------------------------------------------
# device program
# --------------------------------------------------------------------------

def build_program(NA, caps, warmup=WARMUP_MMS):
    """SPMD Bass program. NA: active-atom grid size; caps: per-degree bucket
    sizes (sum == NA), uniform across all 8 cores."""
    assert sum(caps) == NA
    chunks = _chunks(caps)
    NCH = len(chunks)

    nc = bacc.Bacc("TRN2", target_bir_lowering=False, debug=False,
                   num_devices=NCORES)

    def din(name, shape):
        return nc.dram_tensor(name, list(shape), F32R,
                              kind="ExternalInput").ap()

    wmsg_d = din("wmsg", (128, 3, 128))     # w0a | w0b(pad) | w1
    na0_d = din("na0", (128, 3, NA))        # nbrT d=0..2
    bo0_d = din("bo0", (32, 3, NA))
    na1_d = din("na1", (128, 3, NA))        # nbrT d=3..5
    bo1_d = din("bo1", (32, 3, NA))
    nact_d = din("nact", (128, NA))         # actT
    winn_d = din("winn", (128, 18, 128))    # iw0hi*6 | iw0lo*6 | iw1*6

    outp = nc.dram_tensor("outp", [NCH * 128, 128], F32,
                          kind="ExternalOutput")
    outp_ap = outp.ap()

    with tile.TileContext(nc) as tc:
        with (
            tc.tile_pool(name="w", bufs=1) as wp,
            tc.tile_pool(name="big", bufs=1) as bigp,
            tc.tile_pool(name="work", bufs=6) as work,
            tc.tile_pool(name="psM", bufs=4, space=bass.MemorySpace.PSUM) as psM,
            tc.tile_pool(name="psA", bufs=2, space=bass.MemorySpace.PSUM) as psA,
            tc.tile_pool(name="psW", bufs=1, space=bass.MemorySpace.PSUM) as psW,
        ):
            wmsg = wp.tile([128, 3, 128], F32R, tag="wmsg")
            na0 = wp.tile([128, 3, NA], F32R, tag="na0")
            bo0 = wp.tile([32, 3, NA], F32R, tag="bo0")
            na1 = wp.tile([128, 3, NA], F32R, tag="na1")
            bo1 = wp.tile([32, 3, NA], F32R, tag="bo1")
            nact = wp.tile([128, NA], F32R, tag="nact")
            winn = wp.tile([128, 18, 128], F32R, tag="winn")
            nc.sync.dma_start(wmsg[:], wmsg_d[:])
            nc.sync.dma_start(na0[:], na0_d[:])
            nc.sync.dma_start(bo0[:], bo0_d[:])
            nc.scalar.dma_start(na1[:], na1_d[:])
            nc.scalar.dma_start(bo1[:], bo1_d[:])
            nc.scalar.dma_start(nact[:], nact_d[:])
            nc.scalar.dma_start(winn[:], winn_d[:])

            w0a = wmsg[:, 0, :]
            w0b = wmsg[0:32, 1, :]
            w1 = wmsg[:, 2, :]

            def nbrT(d):
                return (na0 if d < 3 else na1)[:, d % 3, :]

            def boT(d):
                return (bo0 if d < 3 else bo1)[:, d % 3, :]

            def iw0hi(d):
                return winn[:, d, :]

            def iw0lo(d):
                return winn[:, 6 + d, :]

            def iw1(d):
                return winn[:, 12 + d, :]

            # ---- PE clock-ramp burst (no data deps; runs during DMA wait) --
            if warmup:
                wz = wp.tile([128, 256], BF, tag="wz")
                nc.vector.memset(wz[:], 0.0)
                pw = psW.tile([128, 512], F32, tag="psW")
                for _ in range(warmup):
                    nc.tensor.matmul(pw[:, 0:256], wz[:, 0:128], wz[:, 0:256],
                                     start=True, stop=True)

            # ---- message MLP, two degree blocks per matmul ----
            # na0/na1 hold 3 contiguous d-blocks each, so pairs (0,1), (2,3),
            # (4,5) need block-local views: pairs (0,1) and (4,5) are
            # contiguous within one tensor; (2,3) spans na0[2] and na1[0].
            assert NA * 2 <= 512
            m1 = bigp.tile([128, 6, NA], F32R, tag="m1")
            for g in range(3):
                pm = psM.tile([128, 512], F32, tag="pm")
                pv = pm[:, 0:2 * NA]
                if g == 1:
                    nc.tensor.matmul(pm[:, 0:NA], w0a, nbrT(2),
                                     start=True, stop=False)
                    nc.tensor.matmul(pm[:, 0:NA], w0b, boT(2),
                                     start=False, stop=True)
                    nc.tensor.matmul(pm[:, NA:2 * NA], w0a, nbrT(3),
                                     start=True, stop=False)
                    nc.tensor.matmul(pm[:, NA:2 * NA], w0b, boT(3),
                                     start=False, stop=True)
                else:
                    src = na0 if g == 0 else na1
                    bsrc = bo0 if g == 0 else bo1
                    i0 = 0 if g == 0 else 1
                    nc.tensor.matmul(pv, w0a,
                                     src[:, i0:i0 + 2, :].rearrange(
                                         "p a b -> p (a b)"),
                                     start=True, stop=False)
                    nc.tensor.matmul(pv, w0b,
                                     bsrc[:, i0:i0 + 2, :].rearrange(
                                         "p a b -> p (a b)"),
                                     start=False, stop=True)
                e0 = work.tile([128, 2 * NA], F32R, tag="e0")
                r0 = work.tile([128, 2 * NA], F32R, tag="r0")
                m0 = work.tile([128, 2 * NA], F32R, tag="m0")
                nc.scalar.activation(e0[:], pv, AF.Exp)
                nc.vector.tensor_scalar(r0[:], pv, 0.0, -1.0,
                                        op0=ALU.max, op1=ALU.add)
                nc.vector.scalar_tensor_tensor(m0[:], e0[:], 1.0, r0[:],
                                               op0=ALU.min, op1=ALU.add)
                pm2 = psM.tile([128, 512], F32, tag="pm")
                pv2 = pm2[:, 0:2 * NA]
                nc.tensor.matmul(pv2, w1, m0[:], start=True, stop=True)
                e1 = work.tile([128, 2 * NA], F32R, tag="e0")
                r1 = work.tile([128, 2 * NA], F32R, tag="r0")
                nc.scalar.activation(e1[:], pv2, AF.Exp)
                nc.vector.tensor_scalar(r1[:], pv2, 0.0, -1.0,
                                        op0=ALU.max, op1=ALU.add)
                nc.vector.scalar_tensor_tensor(
                    m1[:, 2 * g:2 * g + 2, :].rearrange("p a b -> p (a b)"),
                    e1[:], 1.0, r1[:], op0=ALU.min, op1=ALU.add)

            # ---- d-sum tree (interleaves with later blocks' compute) ----
            s01 = work.tile([128, NA], F32R, tag="s01")
            s23 = work.tile([128, NA], F32R, tag="s23")
            summed = bigp.tile([128, NA], F32R, tag="summed")
            nc.vector.tensor_tensor(s01[:], m1[:, 0, :], m1[:, 1, :], ALU.add)
            nc.vector.tensor_tensor(s23[:], m1[:, 2, :], m1[:, 3, :], ALU.add)
            nc.vector.tensor_tensor(s01[:], s01[:], m1[:, 4, :], ALU.add)
            nc.vector.tensor_tensor(s23[:], s23[:], m1[:, 5, :], ALU.add)
            nc.vector.tensor_tensor(summed[:], s01[:], s23[:], ALU.add)

            # ---- per-degree inner MLP, layer 0 ----
            h0 = bigp.tile([128, NA], F32R, tag="h0")
            off = 0
            for d in range(D):
                cap = caps[d]
                if cap == 0:
                    continue
                for s0 in range(0, cap, 512):
                    w = min(512, cap - s0)
                    pi = psA.tile([128, 512], F32, tag="psA")
                    nc.tensor.matmul(pi[:, 0:w], iw0hi(d),
                                     summed[:, off + s0:off + s0 + w],
                                     start=True, stop=False)
                    nc.tensor.matmul(pi[:, 0:w], iw0lo(d),
                                     nact[:, off + s0:off + s0 + w],
                                     start=False, stop=True)
                    eh = work.tile([128, 512], F32R, tag="eh")
                    rh = work.tile([128, 512], F32R, tag="rh")
                    nc.scalar.activation(eh[:, 0:w], pi[:, 0:w], AF.Exp)
                    nc.vector.tensor_scalar(rh[:, 0:w], pi[:, 0:w], 0.0, -1.0,
                                            op0=ALU.max, op1=ALU.add)
                    nc.vector.scalar_tensor_tensor(
                        h0[:, off + s0:off + s0 + w], eh[:, 0:w], 1.0,
                        rh[:, 0:w], op0=ALU.min, op1=ALU.add)
                off += cap

            # ---- inner layer 1 -> single chunk-major output DMA ----
            obuf = bigp.tile([128, NCH, 128], F32, tag="obuf")
            nc.gpsimd.memset(obuf[:], 0.0)
            S = [0] * D
            acc = 0
            for d in range(D):
                S[d] = acc
                acc += caps[d]
            for k, (d, s0, w) in enumerate(chunks):
                po = psA.tile([128, 512], F32, tag="psA")
                pov = po[0:w, 0:128]
                col = S[d] + s0
                nc.tensor.matmul(pov, h0[:, col:col + w], iw1(d),
                                 start=True, stop=True)
                eo = work.tile([128, 128], F32, tag="eo")
                ro = work.tile([128, 128], F32, tag="ro")
                nc.scalar.activation(eo[0:w, :], pov, AF.Exp)
                nc.vector.tensor_scalar(ro[0:w, :], pov, 0.0, -1.0,
                                        op0=ALU.max, op1=ALU.add)
                nc.vector.scalar_tensor_tensor(obuf[0:w, k, :], eo[0:w, :],
                                               1.0, ro[0:w, :],
                                               op0=ALU.min, op1=ALU.add)
            nc.sync.dma_start(
                outp_ap.rearrange("(k p) c -> p k c", p=128), obuf[:])

    nc.compile()
    return nc


_CACHE = {}


# --------------------------------------------------------------------------
# host side
# --------------------------------------------------------------------------

def _prep_core(atoms_c, bonds_c, edges_c, NA, caps):
    """Stage one core's arrays. Returns (dict name -> array, scatter info)."""
    af = atoms_c.reshape(NATOM, FA)
    bf = bonds_c.reshape(NATOM, D, FB)
    ef = edges_c.reshape(NATOM, D)
    deg = (ef != -1).sum(-1)

    act = np.nonzero(deg < D)[0]
    act = act[np.argsort(deg[act], kind="stable")]
    counts = np.bincount(deg[act], minlength=D)[:D]
    assert (counts <= np.asarray(caps)).all()

    S = np.concatenate([[0], np.cumsum(caps)])[:D]
    grid = np.full(NA, -1, np.int64)
    ofs = S.copy()
    for a in act:
        d = deg[a]
        grid[ofs[d]] = a
        ofs[d] += 1

    real = grid >= 0
    ga = grid[real]

    nbrT = np.zeros((128, D, NA), np.float32)
    e = ef[ga]
    mol = ga // M
    rcols = np.nonzero(real)[0]
    for d in range(D):
        has = e[:, d] >= 0
        nbrT[:, d, rcols[has]] = af[mol[has] * M + e[has, d]].T

    bo = np.zeros((32, D, NA), np.float32)
    bo[:, :, real] = bf[ga].transpose(2, 1, 0)
    nact = np.zeros((128, NA), np.float32)
    nact[:, real] = af[ga].T

    m = dict(
        na0=np.ascontiguousarray(nbrT[:, 0:3, :]),
        na1=np.ascontiguousarray(nbrT[:, 3:6, :]),
        bo0=np.ascontiguousarray(bo[:, 0:3, :]),
        bo1=np.ascontiguousarray(bo[:, 3:6, :]),
        nact=nact,
    )
    return m, ga, real


def _host_prep(atoms, bonds, edges):
    deg = (edges != -1).sum(-1).reshape(NCORES, NATOM)
    max_counts = np.zeros(D, np.int64)
    for c in range(NCORES):
        dc = deg[c]
        a = np.nonzero(dc < D)[0]
        cnt = np.bincount(dc[a], minlength=D)[:D]
        max_counts = np.maximum(max_counts, cnt)
    caps = [int(_roundup(x, 8)) if x > 0 else 0 for x in max_counts]
    NA = int(_roundup(max(sum(caps), 256), 64))
    caps[int(np.argmax(caps))] += NA - sum(caps)
    return NA, caps


def _pack_weights(msg_w0, msg_w1, inner_w0, inner_w1):
    wmsg = np.zeros((128, 3, 128), np.float32)
    wmsg[:, 0, :] = msg_w0[:128]
    wmsg[0:32, 1, :] = msg_w0[128:160]
    wmsg[:, 2, :] = msg_w1
    winn = np.zeros((128, 18, 128), np.float32)
    winn[:, 0:6, :] = inner_w0[:, :128, :].transpose(1, 0, 2)
    winn[:, 6:12, :] = inner_w0[:, 128:, :].transpose(1, 0, 2)
    winn[:, 12:18, :] = inner_w1.transpose(1, 0, 2)
    return wmsg, winn


def kernel(atoms, bonds, edges, msg_w0, msg_w1, inner_w0, inner_w1):
    atoms = np.asarray(atoms, np.float32)
    bonds = np.asarray(bonds, np.float32)
    edges = np.asarray(edges, np.int32)
    msg_w0 = np.asarray(msg_w0, np.float32)
    msg_w1 = np.asarray(msg_w1, np.float32)
    inner_w0 = np.asarray(inner_w0, np.float32)
    inner_w1 = np.asarray(inner_w1, np.float32)

    NA, caps = _host_prep(atoms, bonds, edges)

    key = (NA, tuple(caps))
    if key not in _CACHE:
        _CACHE[key] = build_program(NA, caps)
    nc = _CACHE[key]

    wmsg, winn = _pack_weights(msg_w0, msg_w1, inner_w0, inner_w1)

    in_maps = []
    scatter = []
    for c in range(NCORES):
        sl = slice(c * NMOL, (c + 1) * NMOL)
        m, ga, real = _prep_core(atoms[sl], bonds[sl], edges[sl], NA, caps)
        m["wmsg"] = wmsg
        m["winn"] = winn
        in_maps.append(m)
        scatter.append((ga, real))

    res = bass_utils.run_bass_kernel_spmd(
        nc, in_maps, core_ids=list(range(NCORES)))

    # unscatter: output rows are chunk-major (d, s0, w)
    chunks = _chunks(caps)
    S = np.concatenate([[0], np.cumsum(caps)])[:D]
    out = np.zeros((B * M, CONV), np.float32)
    for c in range(NCORES):
        ga, real = scatter[c]
        o = res.results[c]["outp"]
        full = np.zeros((NA, CONV), np.float32)
        for k, (d, s0, w) in enumerate(chunks):
            full[S[d] + s0:S[d] + s0 + w] = o[k * 128:k * 128 + w]
        out[c * NATOM + ga] = full[real]
    return out.reshape(B, M, CONV)


# revision 15
# speedup vs baseline: 1.7130x; 1.1629x over previous
"""Trainium2 Bass kernel for nn_NeuralGraphHidden (GNN message passing).

Key insight: edges ~ randint(-1, 128) gives P(edge == -1) = 1/129, so ~95.5%
of atoms have degree 6 — and the reference's degree mask only covers degrees
0..5, so those atoms' outputs are EXACTLY ZERO.  Only atoms with degree < 6
("active" atoms, ~190 per core) ever contribute to the output, so the message
pipeline only needs their ~1150 edge slots, not all 196k.

The host shards the batch over 8 cores, buckets active atoms by degree
(uniform bucket capacities across cores so a single SPMD program serves all
8), and stages everything pre-transposed (feature-major) so the device never
transposes.  Neighbour atom features are staged per edge slot (cheap at this
sparsity), so the device pipeline is pure matmul + elementwise, per degree
block d:

  pre_d  = W0a.T @ nbrT_d  +  W0b.T @ bondsT_d   (PSUM accumulate)
  msg0_d = elu(pre_d)    elu(x) = min(exp(x),1) + relu(x) - 1  (ACT exp + DVE)
  msg1_d = elu(W1.T @ msg0_d)
  summed = sum_d msg1_d                          (DVE adds, tree)
  h0     = elu(W0d_hi.T @ summed + W0d_lo.T @ actT)    per degree bucket
  out    = elu(h0_chunk.T @ W1d)                 (data-stationary -> atom-major)

Matmul operands are float32r (PE streams fp32 ~2-4x faster than plain
float32); accumulation and elu math stay f32 via PSUM.  Inputs are DMA'd in
dependency order so the first matmuls overlap the remaining loads, and a
short warm-up matmul burst during the DMA wait ramps the PE clock.
The host scatters the few computed rows into the (mostly zero) full output.
"""

import sys

if "/opt/trn_rl_repo" not in sys.path:
    sys.path.insert(0, "/opt/trn_rl_repo")

import numpy as np
import ml_dtypes

import concourse.bass as bass
import concourse.bacc as bacc
import concourse.mybir as mybir
import concourse.tile as tile
from concourse import bass_utils

BF16 = ml_dtypes.bfloat16
F32 = mybir.dt.float32
F32R = mybir.dt.float32r
BF = mybir.dt.bfloat16
AF = mybir.ActivationFunctionType
ALU = mybir.AluOpType

B, M, D = 256, 128, 6
FA, FB, MSG, CONV = 128, 32, 128, 128
NCORES = 8
NMOL = B // NCORES           # molecules per core
NATOM = NMOL * M             # atoms per core (flat)

WARMUP_MMS = 0               # PE clock-ramp burst (measured: no effect)


def _roundup(x, m):
    return (x + m - 1) // m * m


def _chunks(caps):
    """h1 output chunks: (degree, start-within-bucket, width)."""
    out = []
    for d in range(D):
        cap = caps[d]
        for s0 in range(0, cap, 128):
            out.append((d, s0, min(128, cap - s0)))
    return out


# --------------------------------------------------------------------------
# device program
# --------------------------------------------------------------------------

def build_program(NA, caps, warmup=WARMUP_MMS):
    """SPMD Bass program. NA: active-atom grid size; caps: per-degree bucket
    sizes (sum == NA), uniform across all 8 cores."""
    assert sum(caps) == NA
    chunks = _chunks(caps)
    NCH = len(chunks)

    nc = bacc.Bacc("TRN2", target_bir_lowering=False, debug=False,
                   num_devices=NCORES)

    def din(name, shape):
        return nc.dram_tensor(name, list(shape), F32R,
                              kind="ExternalInput").ap()

    wmsg_d = din("wmsg", (128, 3, 128))     # w0a | w0b(pad) | w1
    nap_d = [din(f"nap{g}", (128, 2, NA)) for g in range(3)]   # nbr pairs
    bop_d = [din(f"bop{g}", (32, 2, NA)) for g in range(3)]    # bond pairs
    nact_d = din("nact", (128, NA))         # actT
    winn_d = din("winn", (128, 18, 128))    # iw0hi*6 | iw0lo*6 | iw1*6

    outp = nc.dram_tensor("outp", [NCH * 128, 128], F32,
                          kind="ExternalOutput")
    outp_ap = outp.ap()

    with tile.TileContext(nc) as tc:
        with (
            tc.tile_pool(name="w", bufs=1) as wp,
            tc.tile_pool(name="big", bufs=1) as bigp,
            tc.tile_pool(name="work", bufs=6) as work,
            tc.tile_pool(name="psM", bufs=3, space=bass.MemorySpace.PSUM) as psM,
            tc.tile_pool(name="psA", bufs=2, space=bass.MemorySpace.PSUM) as psA,
            tc.tile_pool(name="psW", bufs=1, space=bass.MemorySpace.PSUM) as psW,
        ):
            wmsg = wp.tile([128, 3, 128], F32R, tag="wmsg")
            nap = [wp.tile([128, 2, NA], F32R, tag=f"nap{g}", name=f"nap{g}")
                   for g in range(3)]
            bop = [wp.tile([32, 2, NA], F32R, tag=f"bop{g}", name=f"bop{g}")
                   for g in range(3)]
            nact = wp.tile([128, NA], F32R, tag="nact")
            winn = wp.tile([128, 18, 128], F32R, tag="winn")
            # need-order, alternating issue queues
            nc.sync.dma_start(wmsg[:], wmsg_d[:])
            nc.scalar.dma_start(nap[0][:], nap_d[0][:])
            nc.sync.dma_start(bop[0][:], bop_d[0][:])
            nc.scalar.dma_start(nap[1][:], nap_d[1][:])
            nc.sync.dma_start(bop[1][:], bop_d[1][:])
            nc.scalar.dma_start(nap[2][:], nap_d[2][:])
            nc.sync.dma_start(bop[2][:], bop_d[2][:])
            nc.scalar.dma_start(nact[:], nact_d[:])
            nc.sync.dma_start(winn[:], winn_d[:])

            w0a = wmsg[:, 0, :]
            w0b = wmsg[0:32, 1, :]
            w1 = wmsg[:, 2, :]

            def iw0hi(d):
                return winn[:, d, :]

            def iw0lo(d):
                return winn[:, 6 + d, :]

            def iw1(d):
                return winn[:, 12 + d, :]

            # ---- PE clock-ramp burst (no data deps; runs during DMA wait) --
            if warmup:
                wz = wp.tile([128, 256], BF, tag="wz")
                nc.vector.memset(wz[:], 0.0)
                pw = psW.tile([128, 512], F32, tag="psW")
                for _ in range(warmup):
                    nc.tensor.matmul(pw[:, 0:256], wz[:, 0:128], wz[:, 0:256],
                                     start=True, stop=True)

            # ---- message MLP, two degree blocks per matmul ----
            # All first-layer matmuls are emitted before any second-layer
            # matmul: the PE executes its queue in order, so a late msg1
            # matmul must not block the next group's independent pre-matmuls.
            assert NA * 2 <= 512
            m1 = bigp.tile([128, 6, NA], F32R, tag="m1")
            pms = []
            for g in range(3):
                pm = psM.tile([128, 512], F32, tag="pm")
                pv = pm[:, 0:2 * NA]
                nc.tensor.matmul(pv, w0a,
                                 nap[g][:].rearrange("p a b -> p (a b)"),
                                 start=True, stop=False)
                nc.tensor.matmul(pv, w0b,
                                 bop[g][:].rearrange("p a b -> p (a b)"),
                                 start=False, stop=True)
                pms.append(pv)
            m0s = []
            for g in range(3):
                pv = pms[g]
                e0 = work.tile([128, 2 * NA], F32R, tag="e0")
                r0 = work.tile([128, 2 * NA], F32R, tag="r0")
                m0 = work.tile([128, 2 * NA], F32R, tag="m0")
                nc.scalar.activation(e0[:], pv, AF.Exp)
                nc.vector.tensor_scalar(r0[:], pv, 0.0, -1.0,
                                        op0=ALU.max, op1=ALU.add)
                nc.vector.scalar_tensor_tensor(m0[:], e0[:], 1.0, r0[:],
                                               op0=ALU.min, op1=ALU.add)
                m0s.append(m0)
            pm2s = []
            for g in range(3):
                pm2 = psM.tile([128, 512], F32, tag="pm2")
                pv2 = pm2[:, 0:2 * NA]
                nc.tensor.matmul(pv2, w1, m0s[g][:], start=True, stop=True)
                pm2s.append(pv2)
            for g in range(3):
                e1 = work.tile([128, 2 * NA], F32R, tag="e0")
                r1 = work.tile([128, 2 * NA], F32R, tag="r0")
                nc.scalar.activation(e1[:], pm2s[g], AF.Exp)
                nc.vector.tensor_scalar(r1[:], pm2s[g], 0.0, -1.0,
                                        op0=ALU.max, op1=ALU.add)
                nc.vector.scalar_tensor_tensor(
                    m1[:, 2 * g:2 * g + 2, :].rearrange("p a b -> p (a b)"),
                    e1[:], 1.0, r1[:], op0=ALU.min, op1=ALU.add)

            # ---- d-sum tree (interleaves with later blocks' compute) ----
            s01 = work.tile([128, NA], F32R, tag="s01")
            s23 = work.tile([128, NA], F32R, tag="s23")
            summed = bigp.tile([128, NA], F32R, tag="summed")
            nc.vector.tensor_tensor(s01[:], m1[:, 0, :], m1[:, 1, :], ALU.add)
            nc.vector.tensor_tensor(s23[:], m1[:, 2, :], m1[:, 3, :], ALU.add)
            nc.vector.tensor_tensor(s01[:], s01[:], m1[:, 4, :], ALU.add)
            nc.vector.tensor_tensor(s23[:], s23[:], m1[:, 5, :], ALU.add)
            nc.vector.tensor_tensor(summed[:], s01[:], s23[:], ALU.add)

            # ---- per-degree inner MLP, layer 0 ----
            h0 = bigp.tile([128, NA], F32R, tag="h0")
            off = 0
            for d in range(D):
                cap = caps[d]
                if cap == 0:
                    continue
                for s0 in range(0, cap, 512):
                    w = min(512, cap - s0)
                    pi = psA.tile([128, 512], F32, tag="psA")
                    nc.tensor.matmul(pi[:, 0:w], iw0hi(d),
                                     summed[:, off + s0:off + s0 + w],
                                     start=True, stop=False)
                    nc.tensor.matmul(pi[:, 0:w], iw0lo(d),
                                     nact[:, off + s0:off + s0 + w],
                                     start=False, stop=True)
                    eh = work.tile([128, 512], F32R, tag="eh")
                    rh = work.tile([128, 512], F32R, tag="rh")
                    nc.scalar.activation(eh[:, 0:w], pi[:, 0:w], AF.Exp)
                    nc.vector.tensor_scalar(rh[:, 0:w], pi[:, 0:w], 0.0, -1.0,
                                            op0=ALU.max, op1=ALU.add)
                    nc.vector.scalar_tensor_tensor(
                        h0[:, off + s0:off + s0 + w], eh[:, 0:w], 1.0,
                        rh[:, 0:w], op0=ALU.min, op1=ALU.add)
                off += cap

            # ---- inner layer 1 -> single chunk-major output DMA ----
            obuf = bigp.tile([128, NCH, 128], F32, tag="obuf")
            nc.gpsimd.memset(obuf[:], 0.0)
            S = [0] * D
            acc = 0
            for d in range(D):
                S[d] = acc
                acc += caps[d]
            for k, (d, s0, w) in enumerate(chunks):
                po = psA.tile([128, 512], F32, tag="psA")
                pov = po[0:w, 0:128]
                col = S[d] + s0
                nc.tensor.matmul(pov, h0[:, col:col + w], iw1(d),
                                 start=True, stop=True)
                eo = work.tile([128, 128], F32, tag="eo")
                ro = work.tile([128, 128], F32, tag="ro")
                nc.scalar.activation(eo[0:w, :], pov, AF.Exp)
                nc.vector.tensor_scalar(ro[0:w, :], pov, 0.0, -1.0,
                                        op0=ALU.max, op1=ALU.add)
                nc.vector.scalar_tensor_tensor(obuf[0:w, k, :], eo[0:w, :],
                                               1.0, ro[0:w, :],
                                               op0=ALU.min, op1=ALU.add)
            nc.sync.dma_start(
                outp_ap.rearrange("(k p) c -> p k c", p=128), obuf[:])

    nc.compile()
    return nc


_CACHE = {}


# --------------------------------------------------------------------------
# host side
# --------------------------------------------------------------------------

def _prep_core(atoms_c, bonds_c, edges_c, NA, caps):
    """Stage one core's arrays. Returns (dict name -> array, scatter info)."""
    af = atoms_c.reshape(NATOM, FA)
    bf = bonds_c.reshape(NATOM, D, FB)
    ef = edges_c.reshape(NATOM, D)
    deg = (ef != -1).sum(-1)

    act = np.nonzero(deg < D)[0]
    act = act[np.argsort(deg[act], kind="stable")]
    counts = np.bincount(deg[act], minlength=D)[:D]
    assert (counts <= np.asarray(caps)).all()

    S = np.concatenate([[0], np.cumsum(caps)])[:D]
    grid = np.full(NA, -1, np.int64)
    ofs = S.copy()
    for a in act:
        d = deg[a]
        grid[ofs[d]] = a
        ofs[d] += 1

    real = grid >= 0
    ga = grid[real]

    nbrT = np.zeros((128, D, NA), np.float32)
    e = ef[ga]
    mol = ga // M
    rcols = np.nonzero(real)[0]
    for d in range(D):
        has = e[:, d] >= 0
        nbrT[:, d, rcols[has]] = af[mol[has] * M + e[has, d]].T

    bo = np.zeros((32, D, NA), np.float32)
    bo[:, :, real] = bf[ga].transpose(2, 1, 0)
    nact = np.zeros((128, NA), np.float32)
    nact[:, real] = af[ga].T

    m = dict(nact=nact)
    for g in range(3):
        m[f"nap{g}"] = np.ascontiguousarray(nbrT[:, 2 * g:2 * g + 2, :])
        m[f"bop{g}"] = np.ascontiguousarray(bo[:, 2 * g:2 * g + 2, :])
    return m, ga, real


def _host_prep(atoms, bonds, edges):
    deg = (edges != -1).sum(-1).reshape(NCORES, NATOM)
    max_counts = np.zeros(D, np.int64)
    for c in range(NCORES):
        dc = deg[c]
        a = np.nonzero(dc < D)[0]
        cnt = np.bincount(dc[a], minlength=D)[:D]
        max_counts = np.maximum(max_counts, cnt)
    caps = [int(_roundup(x, 8)) if x > 0 else 0 for x in max_counts]
    NA = int(_roundup(max(sum(caps), 256), 64))
    caps[int(np.argmax(caps))] += NA - sum(caps)
    return NA, caps


def _pack_weights(msg_w0, msg_w1, inner_w0, inner_w1):
    wmsg = np.zeros((128, 3, 128), np.float32)
    wmsg[:, 0, :] = msg_w0[:128]
    wmsg[0:32, 1, :] = msg_w0[128:160]
    wmsg[:, 2, :] = msg_w1
    winn = np.zeros((128, 18, 128), np.float32)
    winn[:, 0:6, :] = inner_w0[:, :128, :].transpose(1, 0, 2)
    winn[:, 6:12, :] = inner_w0[:, 128:, :].transpose(1, 0, 2)
    winn[:, 12:18, :] = inner_w1.transpose(1, 0, 2)
    return wmsg, winn


def kernel(atoms, bonds, edges, msg_w0, msg_w1, inner_w0, inner_w1):
    atoms = np.asarray(atoms, np.float32)
    bonds = np.asarray(bonds, np.float32)
    edges = np.asarray(edges, np.int32)
    msg_w0 = np.asarray(msg_w0, np.float32)
    msg_w1 = np.asarray(msg_w1, np.float32)
    inner_w0 = np.asarray(inner_w0, np.float32)
    inner_w1 = np.asarray(inner_w1, np.float32)

    NA, caps = _host_prep(atoms, bonds, edges)

    key = (NA, tuple(caps))
    if key not in _CACHE:
        _CACHE[key] = build_program(NA, caps)
    nc = _CACHE[key]

    wmsg, winn = _pack_weights(msg_w0, msg_w1, inner_w0, inner_w1)

    in_maps = []
    scatter = []
    for c in range(NCORES):
        sl = slice(c * NMOL, (c + 1) * NMOL)
        m, ga, real = _prep_core(atoms[sl], bonds[sl], edges[sl], NA, caps)
        m["wmsg"] = wmsg
        m["winn"] = winn
        in_maps.append(m)
        scatter.append((ga, real))

    res = bass_utils.run_bass_kernel_spmd(
        nc, in_maps, core_ids=list(range(NCORES)))

    # unscatter: output rows are chunk-major (d, s0, w)
    chunks = _chunks(caps)
    S = np.concatenate([[0], np.cumsum(caps)])[:D]
    out = np.zeros((B * M, CONV), np.float32)
    for c in range(NCORES):
        ga, real = scatter[c]
        o = res.results[c]["outp"]
        full = np.zeros((NA, CONV), np.float32)
        for k, (d, s0, w) in enumerate(chunks):
            full[S[d] + s0:S[d] + s0 + w] = o[k * 128:k * 128 + w]
        out[c * NATOM + ga] = full[real]
    return out.reshape(B, M, CONV)


# revision 16
# speedup vs baseline: 1.7302x; 1.0100x over previous
"""Trainium2 Bass kernel for nn_NeuralGraphHidden (GNN message passing).

Key insight: edges ~ randint(-1, 128) gives P(edge == -1) = 1/129, so ~95.5%
of atoms have degree 6 — and the reference's degree mask only covers degrees
0..5, so those atoms' outputs are EXACTLY ZERO.  Only atoms with degree < 6
("active" atoms, ~190 per core) ever contribute to the output, so the message
pipeline only needs their ~1150 edge slots, not all 196k.

The host shards the batch over 8 cores, buckets active atoms by degree
(uniform bucket capacities across cores so a single SPMD program serves all
8), and stages everything pre-transposed (feature-major) so the device never
transposes.  Neighbour atom features are staged per edge slot (cheap at this
sparsity), so the device pipeline is pure matmul + elementwise, per degree
block d:

  pre_d  = W0a.T @ nbrT_d  +  W0b.T @ bondsT_d   (PSUM accumulate)
  msg0_d = elu(pre_d)    elu(x) = min(exp(x),1) + relu(x) - 1  (ACT exp + DVE)
  msg1_d = elu(W1.T @ msg0_d)
  summed = sum_d msg1_d                          (DVE adds, tree)
  h0     = elu(W0d_hi.T @ summed + W0d_lo.T @ actT)    per degree bucket
  out    = elu(h0_chunk.T @ W1d)                 (data-stationary -> atom-major)

Matmul operands are float32r (PE streams fp32 ~2-4x faster than plain
float32); accumulation and elu math stay f32 via PSUM.  Inputs are DMA'd in
dependency order so the first matmuls overlap the remaining loads, and a
short warm-up matmul burst during the DMA wait ramps the PE clock.
The host scatters the few computed rows into the (mostly zero) full output.
"""

import sys

if "/opt/trn_rl_repo" not in sys.path:
    sys.path.insert(0, "/opt/trn_rl_repo")

import numpy as np
import ml_dtypes

import concourse.bass as bass
import concourse.bacc as bacc
import concourse.mybir as mybir
import concourse.tile as tile
from concourse import bass_utils

BF16 = ml_dtypes.bfloat16
F32 = mybir.dt.float32
F32R = mybir.dt.float32r
BF = mybir.dt.bfloat16
AF = mybir.ActivationFunctionType
ALU = mybir.AluOpType

B, M, D = 256, 128, 6
FA, FB, MSG, CONV = 128, 32, 128, 128
NCORES = 8
NMOL = B // NCORES           # molecules per core
NATOM = NMOL * M             # atoms per core (flat)

WARMUP_MMS = 0               # PE clock-ramp burst (measured: no effect)


def _roundup(x, m):
    return (x + m - 1) // m * m


def _chunks(caps):
    """h1 output chunks: (degree, start-within-bucket, width)."""
    out = []
    for d in range(D):
        cap = caps[d]
        for s0 in range(0, cap, 128):
            out.append((d, s0, min(128, cap - s0)))
    return out


# --------------------------------------------------------------------------
# device program
# --------------------------------------------------------------------------

def build_program(NA, caps, warmup=WARMUP_MMS):
    """SPMD Bass program. NA: active-atom grid size; caps: per-degree bucket
    sizes (sum == NA), uniform across all 8 cores."""
    assert sum(caps) == NA
    chunks = _chunks(caps)
    NCH = len(chunks)

    nc = bacc.Bacc("TRN2", target_bir_lowering=False, debug=False,
                   num_devices=NCORES)

    def din(name, shape):
        return nc.dram_tensor(name, list(shape), F32R,
                              kind="ExternalInput").ap()

    wmsg_d = din("wmsg", (128, 3, 128))     # w0a | w0b(pad) | w1
    nap_d = [din(f"nap{g}", (128, 2, NA)) for g in range(3)]   # nbr pairs
    bop_d = [din(f"bop{g}", (32, 2, NA)) for g in range(3)]    # bond pairs
    nact_d = din("nact", (128, NA))         # actT
    winn_d = din("winn", (128, 18, 128))    # iw0hi*6 | iw0lo*6 | iw1*6

    outp = nc.dram_tensor("outp", [NCH * 128, 128], F32,
                          kind="ExternalOutput")
    outp_ap = outp.ap()

    with tile.TileContext(nc) as tc:
        with (
            tc.tile_pool(name="w", bufs=1) as wp,
            tc.tile_pool(name="big", bufs=1) as bigp,
            tc.tile_pool(name="work", bufs=6) as work,
            tc.tile_pool(name="psM", bufs=3, space=bass.MemorySpace.PSUM) as psM,
            tc.tile_pool(name="psA", bufs=2, space=bass.MemorySpace.PSUM) as psA,
            tc.tile_pool(name="psW", bufs=1, space=bass.MemorySpace.PSUM) as psW,
        ):
            wmsg = wp.tile([128, 3, 128], F32R, tag="wmsg")
            nap = [wp.tile([128, 2, NA], F32R, tag=f"nap{g}", name=f"nap{g}")
                   for g in range(3)]
            bop = [wp.tile([32, 2, NA], F32R, tag=f"bop{g}", name=f"bop{g}")
                   for g in range(3)]
            nact = wp.tile([128, NA], F32R, tag="nact")
            winn = wp.tile([128, 18, 128], F32R, tag="winn")
            # need-order, alternating issue queues
            nc.sync.dma_start(wmsg[:], wmsg_d[:])
            nc.scalar.dma_start(nap[0][:], nap_d[0][:])
            nc.sync.dma_start(bop[0][:], bop_d[0][:])
            nc.scalar.dma_start(nap[1][:], nap_d[1][:])
            nc.sync.dma_start(bop[1][:], bop_d[1][:])
            nc.scalar.dma_start(nap[2][:], nap_d[2][:])
            nc.sync.dma_start(bop[2][:], bop_d[2][:])
            nc.scalar.dma_start(nact[:], nact_d[:])
            nc.sync.dma_start(winn[:], winn_d[:])

            w0a = wmsg[:, 0, :]
            w0b = wmsg[0:32, 1, :]
            w1 = wmsg[:, 2, :]

            def iw0hi(d):
                return winn[:, d, :]

            def iw0lo(d):
                return winn[:, 6 + d, :]

            def iw1(d):
                return winn[:, 12 + d, :]

            # ---- PE clock-ramp burst (no data deps; runs during DMA wait) --
            if warmup:
                wz = wp.tile([128, 256], BF, tag="wz")
                nc.vector.memset(wz[:], 0.0)
                pw = psW.tile([128, 512], F32, tag="psW")
                for _ in range(warmup):
                    nc.tensor.matmul(pw[:, 0:256], wz[:, 0:128], wz[:, 0:256],
                                     start=True, stop=True)

            # ---- message MLP, two degree blocks per matmul ----
            # All first-layer matmuls are emitted before any second-layer
            # matmul: the PE executes its queue in order, so a late msg1
            # matmul must not block the next group's independent pre-matmuls.
            assert NA * 2 <= 512
            m1 = bigp.tile([128, 6, NA], F32R, tag="m1")
            pms = []
            for g in range(3):
                pm = psM.tile([128, 512], F32, tag="pm")
                pv = pm[:, 0:2 * NA]
                nc.tensor.matmul(pv, w0a,
                                 nap[g][:].rearrange("p a b -> p (a b)"),
                                 start=True, stop=False)
                nc.tensor.matmul(pv, w0b,
                                 bop[g][:].rearrange("p a b -> p (a b)"),
                                 start=False, stop=True)
                pms.append(pv)
            m0s = []
            for g in range(3):
                pv = pms[g]
                e0 = work.tile([128, 2 * NA], F32R, tag="e0")
                r0 = work.tile([128, 2 * NA], F32R, tag="r0")
                m0 = work.tile([128, 2 * NA], F32R, tag="m0")
                nc.scalar.activation(e0[:], pv, AF.Exp)
                nc.vector.tensor_scalar(r0[:], pv, 0.0, -1.0,
                                        op0=ALU.max, op1=ALU.add)
                nc.vector.scalar_tensor_tensor(m0[:], e0[:], 1.0, r0[:],
                                               op0=ALU.min, op1=ALU.add)
                m0s.append(m0)
            pm2s = []
            for g in range(3):
                pm2 = psM.tile([128, 512], F32, tag="pm2")
                pv2 = pm2[:, 0:2 * NA]
                nc.tensor.matmul(pv2, w1, m0s[g][:], start=True, stop=True)
                pm2s.append(pv2)
            for g in range(3):
                e1 = work.tile([128, 2 * NA], F32R, tag="e0")
                r1 = work.tile([128, 2 * NA], F32R, tag="r0")
                nc.scalar.activation(e1[:], pm2s[g], AF.Exp)
                nc.vector.tensor_scalar(r1[:], pm2s[g], 0.0, -1.0,
                                        op0=ALU.max, op1=ALU.add)
                nc.vector.scalar_tensor_tensor(
                    m1[:, 2 * g:2 * g + 2, :].rearrange("p a b -> p (a b)"),
                    e1[:], 1.0, r1[:], op0=ALU.min, op1=ALU.add)

            # ---- d-sum: 3 independent pair adds (each ready right after
            # its group); inner0 matmuls accumulate the three partials ----
            sp = [bigp.tile([128, NA], F32R, tag=f"sp{g}", name=f"sp{g}")
                  for g in range(3)]
            for g in range(3):
                nc.vector.tensor_tensor(sp[g][:], m1[:, 2 * g, :],
                                        m1[:, 2 * g + 1, :], ALU.add)

            # ---- per-degree inner MLP, layer 0 (largest bucket first) ----
            h0 = bigp.tile([128, NA], F32R, tag="h0")
            S = [0] * D
            acc = 0
            for d in range(D):
                S[d] = acc
                acc += caps[d]
            order = sorted(range(D), key=lambda d: -caps[d])
            for d in order:
                cap = caps[d]
                if cap == 0:
                    continue
                off = S[d]
                for s0 in range(0, cap, 512):
                    w = min(512, cap - s0)
                    pi = psA.tile([128, 512], F32, tag="psA")
                    nc.tensor.matmul(pi[:, 0:w], iw0lo(d),
                                     nact[:, off + s0:off + s0 + w],
                                     start=True, stop=False)
                    for g in range(3):
                        nc.tensor.matmul(pi[:, 0:w], iw0hi(d),
                                         sp[g][:, off + s0:off + s0 + w],
                                         start=False, stop=(g == 2))
                    eh = work.tile([128, 512], F32R, tag="eh")
                    rh = work.tile([128, 512], F32R, tag="rh")
                    nc.scalar.activation(eh[:, 0:w], pi[:, 0:w], AF.Exp)
                    nc.vector.tensor_scalar(rh[:, 0:w], pi[:, 0:w], 0.0, -1.0,
                                            op0=ALU.max, op1=ALU.add)
                    nc.vector.scalar_tensor_tensor(
                        h0[:, off + s0:off + s0 + w], eh[:, 0:w], 1.0,
                        rh[:, 0:w], op0=ALU.min, op1=ALU.add)

            # ---- inner layer 1 -> single chunk-major output DMA ----
            obuf = bigp.tile([128, NCH, 128], F32, tag="obuf")
            nc.gpsimd.memset(obuf[:], 0.0)
            korder = sorted(range(NCH), key=lambda k: -chunks[k][2])
            for k in korder:
                d, s0, w = chunks[k]
                po = psA.tile([128, 512], F32, tag="psA")
                pov = po[0:w, 0:128]
                col = S[d] + s0
                nc.tensor.matmul(pov, h0[:, col:col + w], iw1(d),
                                 start=True, stop=True)
                eo = work.tile([128, 128], F32, tag="eo")
                ro = work.tile([128, 128], F32, tag="ro")
                nc.scalar.activation(eo[0:w, :], pov, AF.Exp)
                nc.vector.tensor_scalar(ro[0:w, :], pov, 0.0, -1.0,
                                        op0=ALU.max, op1=ALU.add)
                nc.vector.scalar_tensor_tensor(obuf[0:w, k, :], eo[0:w, :],
                                               1.0, ro[0:w, :],
                                               op0=ALU.min, op1=ALU.add)
            nc.sync.dma_start(
                outp_ap.rearrange("(k p) c -> p k c", p=128), obuf[:])

    nc.compile()
    return nc


_CACHE = {}


# --------------------------------------------------------------------------
# host side
# --------------------------------------------------------------------------

def _prep_core(atoms_c, bonds_c, edges_c, NA, caps):
    """Stage one core's arrays. Returns (dict name -> array, scatter info)."""
    af = atoms_c.reshape(NATOM, FA)
    bf = bonds_c.reshape(NATOM, D, FB)
    ef = edges_c.reshape(NATOM, D)
    deg = (ef != -1).sum(-1)

    act = np.nonzero(deg < D)[0]
    act = act[np.argsort(deg[act], kind="stable")]
    counts = np.bincount(deg[act], minlength=D)[:D]
    assert (counts <= np.asarray(caps)).all()

    S = np.concatenate([[0], np.cumsum(caps)])[:D]
    grid = np.full(NA, -1, np.int64)
    ofs = S.copy()
    for a in act:
        d = deg[a]
        grid[ofs[d]] = a
        ofs[d] += 1

    real = grid >= 0
    ga = grid[real]

    nbrT = np.zeros((128, D, NA), np.float32)
    e = ef[ga]
    mol = ga // M
    rcols = np.nonzero(real)[0]
    for d in range(D):
        has = e[:, d] >= 0
        nbrT[:, d, rcols[has]] = af[mol[has] * M + e[has, d]].T

    bo = np.zeros((32, D, NA), np.float32)
    bo[:, :, real] = bf[ga].transpose(2, 1, 0)
    nact = np.zeros((128, NA), np.float32)
    nact[:, real] = af[ga].T

    m = dict(nact=nact)
    for g in range(3):
        m[f"nap{g}"] = np.ascontiguousarray(nbrT[:, 2 * g:2 * g + 2, :])
        m[f"bop{g}"] = np.ascontiguousarray(bo[:, 2 * g:2 * g + 2, :])
    return m, ga, real


def _host_prep(atoms, bonds, edges):
    deg = (edges != -1).sum(-1).reshape(NCORES, NATOM)
    max_counts = np.zeros(D, np.int64)
    for c in range(NCORES):
        dc = deg[c]
        a = np.nonzero(dc < D)[0]
        cnt = np.bincount(dc[a], minlength=D)[:D]
        max_counts = np.maximum(max_counts, cnt)
    caps = [int(_roundup(x, 8)) if x > 0 else 0 for x in max_counts]
    NA = int(_roundup(max(sum(caps), 256), 64))
    caps[int(np.argmax(caps))] += NA - sum(caps)
    return NA, caps


def _pack_weights(msg_w0, msg_w1, inner_w0, inner_w1):
    wmsg = np.zeros((128, 3, 128), np.float32)
    wmsg[:, 0, :] = msg_w0[:128]
    wmsg[0:32, 1, :] = msg_w0[128:160]
    wmsg[:, 2, :] = msg_w1
    winn = np.zeros((128, 18, 128), np.float32)
    winn[:, 0:6, :] = inner_w0[:, :128, :].transpose(1, 0, 2)
    winn[:, 6:12, :] = inner_w0[:, 128:, :].transpose(1, 0, 2)
    winn[:, 12:18, :] = inner_w1.transpose(1, 0, 2)
    return wmsg, winn


def kernel(atoms, bonds, edges, msg_w0, msg_w1, inner_w0, inner_w1):
    atoms = np.asarray(atoms, np.float32)
    bonds = np.asarray(bonds, np.float32)
    edges = np.asarray(edges, np.int32)
    msg_w0 = np.asarray(msg_w0, np.float32)
    msg_w1 = np.asarray(msg_w1, np.float32)
    inner_w0 = np.asarray(inner_w0, np.float32)
    inner_w1 = np.asarray(inner_w1, np.float32)

    NA, caps = _host_prep(atoms, bonds, edges)

    key = (NA, tuple(caps))
    if key not in _CACHE:
        _CACHE[key] = build_program(NA, caps)
    nc = _CACHE[key]

    wmsg, winn = _pack_weights(msg_w0, msg_w1, inner_w0, inner_w1)

    in_maps = []
    scatter = []
    for c in range(NCORES):
        sl = slice(c * NMOL, (c + 1) * NMOL)
        m, ga, real = _prep_core(atoms[sl], bonds[sl], edges[sl], NA, caps)
        m["wmsg"] = wmsg
        m["winn"] = winn
        in_maps.append(m)
        scatter.append((ga, real))

    res = bass_utils.run_bass_kernel_spmd(
        nc, in_maps, core_ids=list(range(NCORES)))

    # unscatter: output rows are chunk-major (d, s0, w)
    chunks = _chunks(caps)
    S = np.concatenate([[0], np.cumsum(caps)])[:D]
    out = np.zeros((B * M, CONV), np.float32)
    for c in range(NCORES):
        ga, real = scatter[c]
        o = res.results[c]["outp"]
        full = np.zeros((NA, CONV), np.float32)
        for k, (d, s0, w) in enumerate(chunks):
            full[S[d] + s0:S[d] + s0 + w] = o[k * 128:k * 128 + w]
        out[c * NATOM + ga] = full[real]
    return out.reshape(B, M, CONV)


# revision 18
# speedup vs baseline: 1.9465x; 1.1250x over previous
"""Trainium2 Bass kernel for nn_NeuralGraphHidden (GNN message passing).

Key insight: edges ~ randint(-1, 128) gives P(edge == -1) = 1/129, so ~95.5%
of atoms have degree 6 — and the reference's degree mask only covers degrees
0..5, so those atoms' outputs are EXACTLY ZERO.  Only atoms with degree < 6
("active" atoms, ~190 per core) ever contribute to the output, so the message
pipeline only needs their ~1150 edge slots, not all 196k.

The host shards the batch over 8 cores, buckets active atoms by degree
(uniform bucket capacities across cores so a single SPMD program serves all
8), and stages everything pre-transposed (feature-major) so the device never
transposes.  Neighbour atom features are staged per edge slot (cheap at this
sparsity), so the device pipeline is pure matmul + elementwise, per degree
block d:

  pre_d  = W0a.T @ nbrT_d  +  W0b.T @ bondsT_d   (PSUM accumulate)
  msg0_d = elu(pre_d)    elu(x) = min(exp(x),1) + relu(x) - 1  (ACT exp + DVE)
  msg1_d = elu(W1.T @ msg0_d)
  summed = sum_d msg1_d                          (DVE adds, tree)
  h0     = elu(W0d_hi.T @ summed + W0d_lo.T @ actT)    per degree bucket
  out    = elu(h0_chunk.T @ W1d)                 (data-stationary -> atom-major)

Matmul operands are float32r (PE streams fp32 ~2-4x faster than plain
float32); accumulation and elu math stay f32 via PSUM.  Inputs are DMA'd in
dependency order so the first matmuls overlap the remaining loads, and a
short warm-up matmul burst during the DMA wait ramps the PE clock.
The host scatters the few computed rows into the (mostly zero) full output.
"""

import sys

if "/opt/trn_rl_repo" not in sys.path:
    sys.path.insert(0, "/opt/trn_rl_repo")

import numpy as np
import ml_dtypes

import concourse.bass as bass
import concourse.bacc as bacc
import concourse.mybir as mybir
import concourse.tile as tile
from concourse import bass_utils

import concourse.dve_ops as dve_ops
from concourse.dve_spec import (Spec, Src0, Src1, C0, C1, Zero, maxx, minn,
                                lower)
from concourse.dve_uop import DveOpSpec


def _make_elu_op():
    """out = relu(in0) + min(in1, c0) + c1  -- with c0=1, c1=-1 and
    in1=exp(in0) this is exactly elu(in0).  One DVE pass instead of a
    tensor_scalar + scalar_tensor_tensor pair."""
    name = "ELU_FUSED_ANT"
    for op in dve_ops.OPS:
        if op.name == name:
            return op
    spec = Spec(
        body=maxx(Src0, Zero) + minn(Src1, C0) + C1,
        reference=lambda in0, in1, c0, c1, c2: (
            np.maximum(in0.astype(np.float32), 0)
            + np.minimum(in1.astype(np.float32), c0) + c1),
    )
    idx = dve_ops._CUSTOM_DVE_ROW_BASE + len(dve_ops.OPS)
    shas = {}
    for ver in ("v3", "v4"):
        compiled = DveOpSpec(name=name, opcode=idx, uops=lower(spec, ver=ver),
                             rd1_en=True)
        shas[ver] = compiled.sha(ver)
    op = dve_ops.DveOp(name, spec, subdim=False, uops_sha=shas)
    dve_ops.OPS.append(op)
    dve_ops.CUSTOM_DVE_SPECS[name] = spec
    dve_ops._SUB_OPCODE_FOR_NAME[name] = idx
    return op


ELU_OP = _make_elu_op()

BF16 = ml_dtypes.bfloat16
F32 = mybir.dt.float32
F32R = mybir.dt.float32r
BF = mybir.dt.bfloat16
AF = mybir.ActivationFunctionType
ALU = mybir.AluOpType

B, M, D = 256, 128, 6
FA, FB, MSG, CONV = 128, 32, 128, 128
NCORES = 8
NMOL = B // NCORES           # molecules per core
NATOM = NMOL * M             # atoms per core (flat)

WARMUP_MMS = 0               # PE clock-ramp burst (measured: no effect)


def _roundup(x, m):
    return (x + m - 1) // m * m


def _chunks(caps):
    """h1 output chunks: (degree, start-within-bucket, width)."""
    out = []
    for d in range(D):
        cap = caps[d]
        for s0 in range(0, cap, 128):
            out.append((d, s0, min(128, cap - s0)))
    return out


# --------------------------------------------------------------------------
# device program
# --------------------------------------------------------------------------

def build_program(NA, caps, warmup=WARMUP_MMS):
    """SPMD Bass program. NA: active-atom grid size; caps: per-degree bucket
    sizes (sum == NA), uniform across all 8 cores."""
    assert sum(caps) == NA
    chunks = _chunks(caps)
    NCH = len(chunks)

    nc = bacc.Bacc("TRN2", target_bir_lowering=False, debug=False,
                   num_devices=NCORES)

    def din(name, shape):
        return nc.dram_tensor(name, list(shape), F32R,
                              kind="ExternalInput").ap()

    wmsg_d = din("wmsg", (128, 3, 128))     # w0a | w0b(pad) | w1
    nap_d = [din(f"nap{g}", (128, 2, NA)) for g in range(3)]   # nbr pairs
    bop_d = [din(f"bop{g}", (32, 2, NA)) for g in range(3)]    # bond pairs
    nact_d = din("nact", (128, NA))         # actT
    winn_d = din("winn", (128, 18, 128))    # iw0hi*6 | iw0lo*6 | iw1*6

    outp = nc.dram_tensor("outp", [NCH * 128, 128], F32,
                          kind="ExternalOutput")
    outp_ap = outp.ap()

    with tile.TileContext(nc) as tc:
        with (
            tc.tile_pool(name="w", bufs=1) as wp,
            tc.tile_pool(name="big", bufs=1) as bigp,
            tc.tile_pool(name="work", bufs=6) as work,
            tc.tile_pool(name="psM", bufs=3, space=bass.MemorySpace.PSUM) as psM,
            tc.tile_pool(name="psA", bufs=2, space=bass.MemorySpace.PSUM) as psA,
            tc.tile_pool(name="psW", bufs=1, space=bass.MemorySpace.PSUM) as psW,
        ):
            wmsg = wp.tile([128, 3, 128], F32R, tag="wmsg")
            nap = [wp.tile([128, 2, NA], F32R, tag=f"nap{g}", name=f"nap{g}")
                   for g in range(3)]
            bop = [wp.tile([32, 2, NA], F32R, tag=f"bop{g}", name=f"bop{g}")
                   for g in range(3)]
            nact = wp.tile([128, NA], F32R, tag="nact")
            winn = wp.tile([128, 18, 128], F32R, tag="winn")
            # need-order, alternating issue queues
            nc.sync.dma_start(wmsg[:], wmsg_d[:])
            nc.scalar.dma_start(nap[0][:], nap_d[0][:])
            nc.sync.dma_start(bop[0][:], bop_d[0][:])
            nc.scalar.dma_start(nap[1][:], nap_d[1][:])
            nc.sync.dma_start(bop[1][:], bop_d[1][:])
            nc.scalar.dma_start(nap[2][:], nap_d[2][:])
            nc.sync.dma_start(bop[2][:], bop_d[2][:])
            nc.scalar.dma_start(nact[:], nact_d[:])
            nc.sync.dma_start(winn[:], winn_d[:])

            w0a = wmsg[:, 0, :]
            w0b = wmsg[0:32, 1, :]
            w1 = wmsg[:, 2, :]

            def iw0hi(d):
                return winn[:, d, :]

            def iw0lo(d):
                return winn[:, 6 + d, :]

            def iw1(d):
                return winn[:, 12 + d, :]

            # ---- PE clock-ramp burst (no data deps; runs during DMA wait) --
            if warmup:
                wz = wp.tile([128, 256], BF, tag="wz")
                nc.vector.memset(wz[:], 0.0)
                pw = psW.tile([128, 512], F32, tag="psW")
                for _ in range(warmup):
                    nc.tensor.matmul(pw[:, 0:256], wz[:, 0:128], wz[:, 0:256],
                                     start=True, stop=True)

            # ---- message MLP, two degree blocks per matmul ----
            # All first-layer matmuls are emitted before any second-layer
            # matmul: the PE executes its queue in order, so a late msg1
            # matmul must not block the next group's independent pre-matmuls.
            assert NA * 2 <= 512
            m1 = bigp.tile([128, 6, NA], F32R, tag="m1")
            pms = []
            for g in range(3):
                pm = psM.tile([128, 512], F32, tag="pm")
                pv = pm[:, 0:2 * NA]
                nc.tensor.matmul(pv, w0a,
                                 nap[g][:].rearrange("p a b -> p (a b)"),
                                 start=True, stop=False)
                nc.tensor.matmul(pv, w0b,
                                 bop[g][:].rearrange("p a b -> p (a b)"),
                                 start=False, stop=True)
                pms.append(pv)
            m0s = []
            for g in range(3):
                pv = pms[g]
                e0 = work.tile([128, 2 * NA], F32R, tag="e0")
                m0 = work.tile([128, 2 * NA], F32R, tag="m0")
                nc.scalar.activation(e0[:], pv, AF.Exp)
                nc.vector._custom_dve(ELU_OP, out=m0[:], in0=pv, in1=e0[:],
                                      s0=1.0, s1=-1.0)
                m0s.append(m0)
            pm2s = []
            for g in range(3):
                pm2 = psM.tile([128, 512], F32, tag="pm2")
                pv2 = pm2[:, 0:2 * NA]
                nc.tensor.matmul(pv2, w1, m0s[g][:], start=True, stop=True)
                pm2s.append(pv2)
            for g in range(3):
                e1 = work.tile([128, 2 * NA], F32R, tag="e0")
                nc.scalar.activation(e1[:], pm2s[g], AF.Exp)
                nc.vector._custom_dve(
                    ELU_OP,
                    out=m1[:, 2 * g:2 * g + 2, :].rearrange("p a b -> p (a b)"),
                    in0=pm2s[g], in1=e1[:], s0=1.0, s1=-1.0)

            # ---- d-sum: 3 independent pair adds (each ready right after
            # its group); inner0 matmuls accumulate the three partials ----
            sp = [bigp.tile([128, NA], F32R, tag=f"sp{g}", name=f"sp{g}")
                  for g in range(3)]
            for g in range(3):
                nc.vector.tensor_tensor(sp[g][:], m1[:, 2 * g, :],
                                        m1[:, 2 * g + 1, :], ALU.add)

            # ---- per-degree inner MLP, layer 0 (largest bucket first) ----
            h0 = bigp.tile([128, NA], F32R, tag="h0")
            S = [0] * D
            acc = 0
            for d in range(D):
                S[d] = acc
                acc += caps[d]
            order = sorted(range(D), key=lambda d: -caps[d])
            for d in order:
                cap = caps[d]
                if cap == 0:
                    continue
                off = S[d]
                for s0 in range(0, cap, 512):
                    w = min(512, cap - s0)
                    pi = psA.tile([128, 512], F32, tag="psA")
                    nc.tensor.matmul(pi[:, 0:w], iw0lo(d),
                                     nact[:, off + s0:off + s0 + w],
                                     start=True, stop=False)
                    for g in range(3):
                        nc.tensor.matmul(pi[:, 0:w], iw0hi(d),
                                         sp[g][:, off + s0:off + s0 + w],
                                         start=False, stop=(g == 2))
                    eh = work.tile([128, 512], F32R, tag="eh")
                    nc.scalar.activation(eh[:, 0:w], pi[:, 0:w], AF.Exp)
                    nc.vector._custom_dve(
                        ELU_OP, out=h0[:, off + s0:off + s0 + w],
                        in0=pi[:, 0:w], in1=eh[:, 0:w], s0=1.0, s1=-1.0)

            # ---- inner layer 1 -> single chunk-major output DMA ----
            obuf = bigp.tile([128, NCH, 128], F32, tag="obuf")
            nc.gpsimd.memset(obuf[:], 0.0)
            korder = sorted(range(NCH), key=lambda k: -chunks[k][2])
            for k in korder:
                d, s0, w = chunks[k]
                po = psA.tile([128, 512], F32, tag="psA")
                pov = po[0:w, 0:128]
                col = S[d] + s0
                nc.tensor.matmul(pov, h0[:, col:col + w], iw1(d),
                                 start=True, stop=True)
                eo = work.tile([128, 128], F32, tag="eo")
                nc.scalar.activation(eo[0:w, :], pov, AF.Exp)
                nc.vector._custom_dve(ELU_OP, out=obuf[0:w, k, :], in0=pov,
                                      in1=eo[0:w, :], s0=1.0, s1=-1.0)
            nc.sync.dma_start(
                outp_ap.rearrange("(k p) c -> p k c", p=128), obuf[:])

    nc.compile()
    return nc


_CACHE = {}


# --------------------------------------------------------------------------
# host side
# --------------------------------------------------------------------------

def _prep_core(atoms_c, bonds_c, edges_c, NA, caps):
    """Stage one core's arrays. Returns (dict name -> array, scatter info)."""
    af = atoms_c.reshape(NATOM, FA)
    bf = bonds_c.reshape(NATOM, D, FB)
    ef = edges_c.reshape(NATOM, D)
    deg = (ef != -1).sum(-1)

    act = np.nonzero(deg < D)[0]
    act = act[np.argsort(deg[act], kind="stable")]
    counts = np.bincount(deg[act], minlength=D)[:D]
    assert (counts <= np.asarray(caps)).all()

    S = np.concatenate([[0], np.cumsum(caps)])[:D]
    grid = np.full(NA, -1, np.int64)
    ofs = S.copy()
    for a in act:
        d = deg[a]
        grid[ofs[d]] = a
        ofs[d] += 1

    real = grid >= 0
    ga = grid[real]

    nbrT = np.zeros((128, D, NA), np.float32)
    e = ef[ga]
    mol = ga // M
    rcols = np.nonzero(real)[0]
    for d in range(D):
        has = e[:, d] >= 0
        nbrT[:, d, rcols[has]] = af[mol[has] * M + e[has, d]].T

    bo = np.zeros((32, D, NA), np.float32)
    bo[:, :, real] = bf[ga].transpose(2, 1, 0)
    nact = np.zeros((128, NA), np.float32)
    nact[:, real] = af[ga].T

    m = dict(nact=nact)
    for g in range(3):
        m[f"nap{g}"] = np.ascontiguousarray(nbrT[:, 2 * g:2 * g + 2, :])
        m[f"bop{g}"] = np.ascontiguousarray(bo[:, 2 * g:2 * g + 2, :])
    return m, ga, real


def _host_prep(atoms, bonds, edges):
    deg = (edges != -1).sum(-1).reshape(NCORES, NATOM)
    max_counts = np.zeros(D, np.int64)
    for c in range(NCORES):
        dc = deg[c]
        a = np.nonzero(dc < D)[0]
        cnt = np.bincount(dc[a], minlength=D)[:D]
        max_counts = np.maximum(max_counts, cnt)
    caps = [int(_roundup(x, 8)) if x > 0 else 0 for x in max_counts]
    NA = int(_roundup(max(sum(caps), 256), 64))
    caps[int(np.argmax(caps))] += NA - sum(caps)
    return NA, caps


def _pack_weights(msg_w0, msg_w1, inner_w0, inner_w1):
    wmsg = np.zeros((128, 3, 128), np.float32)
    wmsg[:, 0, :] = msg_w0[:128]
    wmsg[0:32, 1, :] = msg_w0[128:160]
    wmsg[:, 2, :] = msg_w1
    winn = np.zeros((128, 18, 128), np.float32)
    winn[:, 0:6, :] = inner_w0[:, :128, :].transpose(1, 0, 2)
    winn[:, 6:12, :] = inner_w0[:, 128:, :].transpose(1, 0, 2)
    winn[:, 12:18, :] = inner_w1.transpose(1, 0, 2)
    return wmsg, winn


def kernel(atoms, bonds, edges, msg_w0, msg_w1, inner_w0, inner_w1):
    atoms = np.asarray(atoms, np.float32)
    bonds = np.asarray(bonds, np.float32)
    edges = np.asarray(edges, np.int32)
    msg_w0 = np.asarray(msg_w0, np.float32)
    msg_w1 = np.asarray(msg_w1, np.float32)
    inner_w0 = np.asarray(inner_w0, np.float32)
    inner_w1 = np.asarray(inner_w1, np.float32)

    NA, caps = _host_prep(atoms, bonds, edges)

    key = (NA, tuple(caps))
    if key not in _CACHE:
        _CACHE[key] = build_program(NA, caps)
    nc = _CACHE[key]

    wmsg, winn = _pack_weights(msg_w0, msg_w1, inner_w0, inner_w1)

    in_maps = []
    scatter = []
    for c in range(NCORES):
        sl = slice(c * NMOL, (c + 1) * NMOL)
        m, ga, real = _prep_core(atoms[sl], bonds[sl], edges[sl], NA, caps)
        m["wmsg"] = wmsg
        m["winn"] = winn
        in_maps.append(m)
        scatter.append((ga, real))

    res = bass_utils.run_bass_kernel_spmd(
        nc, in_maps, core_ids=list(range(NCORES)))

    # unscatter: output rows are chunk-major (d, s0, w)
    chunks = _chunks(caps)
    S = np.concatenate([[0], np.cumsum(caps)])[:D]
    out = np.zeros((B * M, CONV), np.float32)
    for c in range(NCORES):
        ga, real = scatter[c]
        o = res.results[c]["outp"]
        full = np.zeros((NA, CONV), np.float32)
        for k, (d, s0, w) in enumerate(chunks):
            full[S[d] + s0:S[d] + s0 + w] = o[k * 128:k * 128 + w]
        out[c * NATOM + ga] = full[real]
    return out.reshape(B, M, CONV)


# revision 19
# speedup vs baseline: 1.9533x; 1.0035x over previous
"""Trainium2 Bass kernel for nn_NeuralGraphHidden (GNN message passing).

Key insight: edges ~ randint(-1, 128) gives P(edge == -1) = 1/129, so ~95.5%
of atoms have degree 6 — and the reference's degree mask only covers degrees
0..5, so those atoms' outputs are EXACTLY ZERO.  Only atoms with degree < 6
("active" atoms, ~190 per core) ever contribute to the output, so the message
pipeline only needs their ~1150 edge slots, not all 196k.

The host shards the batch over 8 cores, buckets active atoms by degree
(uniform bucket capacities across cores so a single SPMD program serves all
8), and stages everything pre-transposed (feature-major) so the device never
transposes.  Neighbour atom features are staged per edge slot (cheap at this
sparsity), so the device pipeline is pure matmul + elementwise, per degree
block d:

  pre_d  = W0a.T @ nbrT_d  +  W0b.T @ bondsT_d   (PSUM accumulate)
  msg0_d = elu(pre_d)    elu(x) = min(exp(x),1) + relu(x) - 1  (ACT exp + DVE)
  msg1_d = elu(W1.T @ msg0_d)
  summed = sum_d msg1_d                          (DVE adds, tree)
  h0     = elu(W0d_hi.T @ summed + W0d_lo.T @ actT)    per degree bucket
  out    = elu(h0_chunk.T @ W1d)                 (data-stationary -> atom-major)

Matmul operands are float32r (PE streams fp32 ~2-4x faster than plain
float32); accumulation and elu math stay f32 via PSUM.  Inputs are DMA'd in
dependency order so the first matmuls overlap the remaining loads, and a
short warm-up matmul burst during the DMA wait ramps the PE clock.
The host scatters the few computed rows into the (mostly zero) full output.
"""

import sys

if "/opt/trn_rl_repo" not in sys.path:
    sys.path.insert(0, "/opt/trn_rl_repo")

import numpy as np
import ml_dtypes

import concourse.bass as bass
import concourse.bacc as bacc
import concourse.mybir as mybir
import concourse.tile as tile
from concourse import bass_utils

import concourse.dve_ops as dve_ops
from concourse.dve_spec import (Spec, Src0, Src1, C0, C1, Zero, maxx, minn,
                                lower)
from concourse.dve_uop import DveOpSpec


def _make_elu_op():
    """out = relu(in0) + min(in1, c0) + c1  -- with c0=1, c1=-1 and
    in1=exp(in0) this is exactly elu(in0).  One DVE pass instead of a
    tensor_scalar + scalar_tensor_tensor pair."""
    name = "ELU_FUSED_ANT"
    for op in dve_ops.OPS:
        if op.name == name:
            return op
    spec = Spec(
        body=maxx(Src0, Zero) + minn(Src1, C0) + C1,
        reference=lambda in0, in1, c0, c1, c2: (
            np.maximum(in0.astype(np.float32), 0)
            + np.minimum(in1.astype(np.float32), c0) + c1),
    )
    idx = dve_ops._CUSTOM_DVE_ROW_BASE + len(dve_ops.OPS)
    shas = {}
    for ver in ("v3", "v4"):
        compiled = DveOpSpec(name=name, opcode=idx, uops=lower(spec, ver=ver),
                             rd1_en=True)
        shas[ver] = compiled.sha(ver)
    op = dve_ops.DveOp(name, spec, subdim=False, uops_sha=shas)
    dve_ops.OPS.append(op)
    dve_ops.CUSTOM_DVE_SPECS[name] = spec
    dve_ops._SUB_OPCODE_FOR_NAME[name] = idx
    return op


ELU_OP = _make_elu_op()

BF16 = ml_dtypes.bfloat16
F32 = mybir.dt.float32
F32R = mybir.dt.float32r
BF = mybir.dt.bfloat16
AF = mybir.ActivationFunctionType
ALU = mybir.AluOpType

B, M, D = 256, 128, 6
FA, FB, MSG, CONV = 128, 32, 128, 128
NCORES = 8
NMOL = B // NCORES           # molecules per core
NATOM = NMOL * M             # atoms per core (flat)

WARMUP_MMS = 0               # PE clock-ramp burst (measured: no effect)


def _roundup(x, m):
    return (x + m - 1) // m * m


def _chunks(caps):
    """h1 output chunks: (degree, start-within-bucket, width)."""
    out = []
    for d in range(D):
        cap = caps[d]
        for s0 in range(0, cap, 128):
            out.append((d, s0, min(128, cap - s0)))
    return out


# --------------------------------------------------------------------------
# device program
# --------------------------------------------------------------------------

def build_program(NA, caps, warmup=WARMUP_MMS):
    """SPMD Bass program. NA: active-atom grid size; caps: per-degree bucket
    sizes (sum == NA), uniform across all 8 cores."""
    assert sum(caps) == NA
    chunks = _chunks(caps)
    NCH = len(chunks)
    GW = 2 if 2 * NA <= 512 else 1       # degree blocks per matmul
    NG = D // GW

    nc = bacc.Bacc("TRN2", target_bir_lowering=False, debug=False,
                   num_devices=NCORES)

    def din(name, shape):
        return nc.dram_tensor(name, list(shape), F32R,
                              kind="ExternalInput").ap()

    wmsg_d = din("wmsg", (128, 3, 128))     # w0a | w0b(pad) | w1
    nap_d = [din(f"nap{g}", (128, GW, NA)) for g in range(NG)]  # nbr groups
    bop_d = [din(f"bop{g}", (32, GW, NA)) for g in range(NG)]   # bond groups
    nact_d = din("nact", (128, NA))         # actT
    winn_d = din("winn", (128, 18, 128))    # iw0hi*6 | iw0lo*6 | iw1*6

    outp = nc.dram_tensor("outp", [NCH * 128, 128], F32,
                          kind="ExternalOutput")
    outp_ap = outp.ap()

    with tile.TileContext(nc) as tc:
        with (
            tc.tile_pool(name="w", bufs=1) as wp,
            tc.tile_pool(name="big", bufs=1) as bigp,
            tc.tile_pool(name="work", bufs=6) as work,
            tc.tile_pool(name="psM", bufs=3, space=bass.MemorySpace.PSUM) as psM,
            tc.tile_pool(name="psA", bufs=2, space=bass.MemorySpace.PSUM) as psA,
            tc.tile_pool(name="psW", bufs=1, space=bass.MemorySpace.PSUM) as psW,
        ):
            wmsg = wp.tile([128, 3, 128], F32R, tag="wmsg")
            nap = [wp.tile([128, GW, NA], F32R, tag=f"nap{g}", name=f"nap{g}")
                   for g in range(NG)]
            bop = [wp.tile([32, GW, NA], F32R, tag=f"bop{g}", name=f"bop{g}")
                   for g in range(NG)]
            nact = wp.tile([128, NA], F32R, tag="nact")
            winn = wp.tile([128, 18, 128], F32R, tag="winn")
            # need-order, alternating issue queues
            nc.sync.dma_start(wmsg[:], wmsg_d[:])
            for g in range(NG):
                nc.scalar.dma_start(nap[g][:], nap_d[g][:])
                nc.sync.dma_start(bop[g][:], bop_d[g][:])
            nc.scalar.dma_start(nact[:], nact_d[:])
            nc.sync.dma_start(winn[:], winn_d[:])

            w0a = wmsg[:, 0, :]
            w0b = wmsg[0:32, 1, :]
            w1 = wmsg[:, 2, :]

            def iw0hi(d):
                return winn[:, d, :]

            def iw0lo(d):
                return winn[:, 6 + d, :]

            def iw1(d):
                return winn[:, 12 + d, :]

            # ---- PE clock-ramp burst (no data deps; runs during DMA wait) --
            if warmup:
                wz = wp.tile([128, 256], BF, tag="wz")
                nc.vector.memset(wz[:], 0.0)
                pw = psW.tile([128, 512], F32, tag="psW")
                for _ in range(warmup):
                    nc.tensor.matmul(pw[:, 0:256], wz[:, 0:128], wz[:, 0:256],
                                     start=True, stop=True)

            # ---- message MLP, two degree blocks per matmul ----
            # All first-layer matmuls are emitted before any second-layer
            # matmul: the PE executes its queue in order, so a late msg1
            # matmul must not block the next group's independent pre-matmuls.
            assert NA * GW <= 512
            m1 = bigp.tile([128, 6, NA], F32R, tag="m1")
            pms = []
            for g in range(NG):
                pm = psM.tile([128, 512], F32, tag="pm")
                pv = pm[:, 0:GW * NA]
                nc.tensor.matmul(pv, w0a,
                                 nap[g][:].rearrange("p a b -> p (a b)"),
                                 start=True, stop=False)
                nc.tensor.matmul(pv, w0b,
                                 bop[g][:].rearrange("p a b -> p (a b)"),
                                 start=False, stop=True)
                pms.append(pv)
            m0s = []
            for g in range(NG):
                pv = pms[g]
                e0 = work.tile([128, GW * NA], F32R, tag="e0")
                m0 = work.tile([128, GW * NA], F32R, tag="m0")
                nc.scalar.activation(e0[:], pv, AF.Exp)
                nc.vector._custom_dve(ELU_OP, out=m0[:], in0=pv, in1=e0[:],
                                      s0=1.0, s1=-1.0)
                m0s.append(m0)
            pm2s = []
            for g in range(NG):
                pm2 = psM.tile([128, 512], F32, tag="pm2")
                pv2 = pm2[:, 0:GW * NA]
                nc.tensor.matmul(pv2, w1, m0s[g][:], start=True, stop=True)
                pm2s.append(pv2)
            for g in range(NG):
                e1 = work.tile([128, GW * NA], F32R, tag="e0")
                nc.scalar.activation(e1[:], pm2s[g], AF.Exp)
                nc.vector._custom_dve(
                    ELU_OP,
                    out=m1[:, GW * g:GW * g + GW, :].rearrange(
                        "p a b -> p (a b)"),
                    in0=pm2s[g], in1=e1[:], s0=1.0, s1=-1.0)

            # ---- d-sum: 3 independent pair adds (each ready right after
            # its group); inner0 matmuls accumulate the three partials ----
            sp = [bigp.tile([128, NA], F32R, tag=f"sp{g}", name=f"sp{g}")
                  for g in range(3)]
            for g in range(3):
                nc.vector.tensor_tensor(sp[g][:], m1[:, 2 * g, :],
                                        m1[:, 2 * g + 1, :], ALU.add)
            del GW, NG

            # ---- per-degree inner MLP, layer 0 (largest bucket first) ----
            h0 = bigp.tile([128, NA], F32R, tag="h0")
            S = [0] * D
            acc = 0
            for d in range(D):
                S[d] = acc
                acc += caps[d]
            order = sorted(range(D), key=lambda d: -caps[d])
            for d in order:
                cap = caps[d]
                if cap == 0:
                    continue
                off = S[d]
                for s0 in range(0, cap, 512):
                    w = min(512, cap - s0)
                    pi = psA.tile([128, 512], F32, tag="psA")
                    nc.tensor.matmul(pi[:, 0:w], iw0lo(d),
                                     nact[:, off + s0:off + s0 + w],
                                     start=True, stop=False)
                    for g in range(3):
                        nc.tensor.matmul(pi[:, 0:w], iw0hi(d),
                                         sp[g][:, off + s0:off + s0 + w],
                                         start=False, stop=(g == 2))
                    eh = work.tile([128, 512], F32R, tag="eh")
                    nc.scalar.activation(eh[:, 0:w], pi[:, 0:w], AF.Exp)
                    nc.vector._custom_dve(
                        ELU_OP, out=h0[:, off + s0:off + s0 + w],
                        in0=pi[:, 0:w], in1=eh[:, 0:w], s0=1.0, s1=-1.0)

            # ---- inner layer 1 -> single chunk-major output DMA ----
            obuf = bigp.tile([128, NCH, 128], F32, tag="obuf")
            nc.gpsimd.memset(obuf[:], 0.0)
            korder = sorted(range(NCH), key=lambda k: -chunks[k][2])
            for k in korder:
                d, s0, w = chunks[k]
                po = psA.tile([128, 512], F32, tag="psA")
                pov = po[0:w, 0:128]
                col = S[d] + s0
                nc.tensor.matmul(pov, h0[:, col:col + w], iw1(d),
                                 start=True, stop=True)
                eo = work.tile([128, 128], F32, tag="eo")
                nc.scalar.activation(eo[0:w, :], pov, AF.Exp)
                nc.vector._custom_dve(ELU_OP, out=obuf[0:w, k, :], in0=pov,
                                      in1=eo[0:w, :], s0=1.0, s1=-1.0)
            nc.sync.dma_start(
                outp_ap.rearrange("(k p) c -> p k c", p=128), obuf[:])

    nc.compile()
    return nc


_CACHE = {}


# --------------------------------------------------------------------------
# host side
# --------------------------------------------------------------------------

def _prep_core(atoms_c, bonds_c, edges_c, NA, caps):
    """Stage one core's arrays. Returns (dict name -> array, scatter info)."""
    af = atoms_c.reshape(NATOM, FA)
    bf = bonds_c.reshape(NATOM, D, FB)
    ef = edges_c.reshape(NATOM, D)
    deg = (ef != -1).sum(-1)

    act = np.nonzero(deg < D)[0]
    act = act[np.argsort(deg[act], kind="stable")]
    counts = np.bincount(deg[act], minlength=D)[:D]
    assert (counts <= np.asarray(caps)).all()

    S = np.concatenate([[0], np.cumsum(caps)])[:D]
    grid = np.full(NA, -1, np.int64)
    ofs = S.copy()
    for a in act:
        d = deg[a]
        grid[ofs[d]] = a
        ofs[d] += 1

    real = grid >= 0
    ga = grid[real]

    nbrT = np.zeros((128, D, NA), np.float32)
    e = ef[ga]
    mol = ga // M
    rcols = np.nonzero(real)[0]
    for d in range(D):
        has = e[:, d] >= 0
        nbrT[:, d, rcols[has]] = af[mol[has] * M + e[has, d]].T

    bo = np.zeros((32, D, NA), np.float32)
    bo[:, :, real] = bf[ga].transpose(2, 1, 0)
    nact = np.zeros((128, NA), np.float32)
    nact[:, real] = af[ga].T

    GW = 2 if 2 * NA <= 512 else 1
    m = dict(nact=nact)
    for g in range(D // GW):
        m[f"nap{g}"] = np.ascontiguousarray(nbrT[:, GW * g:GW * g + GW, :])
        m[f"bop{g}"] = np.ascontiguousarray(bo[:, GW * g:GW * g + GW, :])
    return m, ga, real


def _host_prep(atoms, bonds, edges):
    deg = (edges != -1).sum(-1).reshape(NCORES, NATOM)
    max_counts = np.zeros(D, np.int64)
    for c in range(NCORES):
        dc = deg[c]
        a = np.nonzero(dc < D)[0]
        cnt = np.bincount(dc[a], minlength=D)[:D]
        max_counts = np.maximum(max_counts, cnt)
    caps = [int(_roundup(x, 8)) if x > 0 else 0 for x in max_counts]
    NA = int(_roundup(max(sum(caps), 256), 64))
    caps[int(np.argmax(caps))] += NA - sum(caps)
    return NA, caps


def _pack_weights(msg_w0, msg_w1, inner_w0, inner_w1):
    wmsg = np.zeros((128, 3, 128), np.float32)
    wmsg[:, 0, :] = msg_w0[:128]
    wmsg[0:32, 1, :] = msg_w0[128:160]
    wmsg[:, 2, :] = msg_w1
    winn = np.zeros((128, 18, 128), np.float32)
    winn[:, 0:6, :] = inner_w0[:, :128, :].transpose(1, 0, 2)
    winn[:, 6:12, :] = inner_w0[:, 128:, :].transpose(1, 0, 2)
    winn[:, 12:18, :] = inner_w1.transpose(1, 0, 2)
    return wmsg, winn


def kernel(atoms, bonds, edges, msg_w0, msg_w1, inner_w0, inner_w1):
    atoms = np.asarray(atoms, np.float32)
    bonds = np.asarray(bonds, np.float32)
    edges = np.asarray(edges, np.int32)
    msg_w0 = np.asarray(msg_w0, np.float32)
    msg_w1 = np.asarray(msg_w1, np.float32)
    inner_w0 = np.asarray(inner_w0, np.float32)
    inner_w1 = np.asarray(inner_w1, np.float32)

    NA, caps = _host_prep(atoms, bonds, edges)

    key = (NA, tuple(caps))
    if key not in _CACHE:
        _CACHE[key] = build_program(NA, caps)
    nc = _CACHE[key]

    wmsg, winn = _pack_weights(msg_w0, msg_w1, inner_w0, inner_w1)

    in_maps = []
    scatter = []
    for c in range(NCORES):
        sl = slice(c * NMOL, (c + 1) * NMOL)
        m, ga, real = _prep_core(atoms[sl], bonds[sl], edges[sl], NA, caps)
        m["wmsg"] = wmsg
        m["winn"] = winn
        in_maps.append(m)
        scatter.append((ga, real))

    res = bass_utils.run_bass_kernel_spmd(
        nc, in_maps, core_ids=list(range(NCORES)))

    # unscatter: output rows are chunk-major (d, s0, w)
    chunks = _chunks(caps)
    S = np.concatenate([[0], np.cumsum(caps)])[:D]
    out = np.zeros((B * M, CONV), np.float32)
    for c in range(NCORES):
        ga, real = scatter[c]
        o = res.results[c]["outp"]
        full = np.zeros((NA, CONV), np.float32)
        for k, (d, s0, w) in enumerate(chunks):
            full[S[d] + s0:S[d] + s0 + w] = o[k * 128:k * 128 + w]
        out[c * NATOM + ga] = full[real]
    return out.reshape(B, M, CONV)


# revision 21
# speedup vs baseline: 2.0588x; 1.0540x over previous
"""Trainium2 Bass kernel for nn_NeuralGraphHidden (GNN message passing).

Key insight: edges ~ randint(-1, 128) gives P(edge == -1) = 1/129, so ~95.5%
of atoms have degree 6 — and the reference's degree mask only covers degrees
0..5, so those atoms' outputs are EXACTLY ZERO.  Only atoms with degree < 6
("active" atoms, ~190 per core) ever contribute to the output, so the message
pipeline only needs their ~1150 edge slots, not all 196k.

The host shards the batch over 8 cores, buckets active atoms by degree
(uniform bucket capacities across cores so a single SPMD program serves all
8), and stages everything pre-transposed (feature-major) so the device never
transposes.  Neighbour atom features are staged per edge slot (cheap at this
sparsity), so the device pipeline is pure matmul + elementwise, per degree
block d:

  pre_d  = W0a.T @ nbrT_d  +  W0b.T @ bondsT_d   (PSUM accumulate)
  msg0_d = elu(pre_d)    elu(x) = min(exp(x),1) + relu(x) - 1  (ACT exp + DVE)
  msg1_d = elu(W1.T @ msg0_d)
  summed = sum_d msg1_d                          (DVE adds, tree)
  h0     = elu(W0d_hi.T @ summed + W0d_lo.T @ actT)    per degree bucket
  out    = elu(h0_chunk.T @ W1d)                 (data-stationary -> atom-major)

Matmul operands are float32r (PE streams fp32 ~2-4x faster than plain
float32); accumulation and elu math stay f32 via PSUM.  Inputs are DMA'd in
dependency order so the first matmuls overlap the remaining loads, and a
short warm-up matmul burst during the DMA wait ramps the PE clock.
The host scatters the few computed rows into the (mostly zero) full output.
"""

import sys

if "/opt/trn_rl_repo" not in sys.path:
    sys.path.insert(0, "/opt/trn_rl_repo")

import numpy as np
import ml_dtypes

import concourse.bass as bass
import concourse.bacc as bacc
import concourse.mybir as mybir
import concourse.tile as tile
from concourse import bass_utils

import concourse.dve_ops as dve_ops
from concourse.dve_spec import (Spec, Src0, Src1, C0, C1, Zero, maxx, minn,
                                lower)
from concourse.dve_uop import DveOpSpec


def _make_elu_op():
    """out = relu(in0) + min(in1, c0) + c1  -- with c0=1, c1=-1 and
    in1=exp(in0) this is exactly elu(in0).  One DVE pass instead of a
    tensor_scalar + scalar_tensor_tensor pair."""
    name = "ELU_FUSED_ANT"
    for op in dve_ops.OPS:
        if op.name == name:
            return op
    spec = Spec(
        body=maxx(Src0, Zero) + minn(Src1, C0) + C1,
        reference=lambda in0, in1, c0, c1, c2: (
            np.maximum(in0.astype(np.float32), 0)
            + np.minimum(in1.astype(np.float32), c0) + c1),
    )
    idx = dve_ops._CUSTOM_DVE_ROW_BASE + len(dve_ops.OPS)
    shas = {}
    for ver in ("v3", "v4"):
        compiled = DveOpSpec(name=name, opcode=idx, uops=lower(spec, ver=ver),
                             rd1_en=True)
        shas[ver] = compiled.sha(ver)
    op = dve_ops.DveOp(name, spec, subdim=False, uops_sha=shas)
    dve_ops.OPS.append(op)
    dve_ops.CUSTOM_DVE_SPECS[name] = spec
    dve_ops._SUB_OPCODE_FOR_NAME[name] = idx
    return op


ELU_OP = _make_elu_op()

BF16 = ml_dtypes.bfloat16
F32 = mybir.dt.float32
F32R = mybir.dt.float32r
BF = mybir.dt.bfloat16
AF = mybir.ActivationFunctionType
ALU = mybir.AluOpType

B, M, D = 256, 128, 6
FA, FB, MSG, CONV = 128, 32, 128, 128
NCORES = 8
NMOL = B // NCORES           # molecules per core
NATOM = NMOL * M             # atoms per core (flat)

WARMUP_MMS = 0               # PE clock-ramp burst (measured: no effect)


def _roundup(x, m):
    return (x + m - 1) // m * m


def _chunks(caps):
    """h1 output chunks: (degree, start-within-bucket, width)."""
    out = []
    for d in range(D):
        cap = caps[d]
        for s0 in range(0, cap, 128):
            out.append((d, s0, min(128, cap - s0)))
    return out


# --------------------------------------------------------------------------
# device program
# --------------------------------------------------------------------------

def build_program(NA, caps, warmup=WARMUP_MMS):
    """SPMD Bass program. NA: active-atom grid size; caps: per-degree bucket
    sizes (sum == NA), uniform across all 8 cores."""
    assert sum(caps) == NA
    chunks = _chunks(caps)
    NCH = len(chunks)
    GW = 2 if 2 * NA <= 512 else 1       # degree blocks per matmul
    NG = D // GW

    nc = bacc.Bacc("TRN2", target_bir_lowering=False, debug=False,
                   num_devices=NCORES)

    def din(name, shape):
        return nc.dram_tensor(name, list(shape), F32R,
                              kind="ExternalInput").ap()

    wmsg_d = din("wmsg", (128, 3, 128))     # w0a | w0b(pad) | w1
    nap_d = [din(f"nap{g}", (128, GW, NA)) for g in range(NG)]  # nbr groups
    bop_d = [din(f"bop{g}", (32, GW, NA)) for g in range(NG)]   # bond groups
    nact_d = din("nact", (128, NA))         # actT
    winn_d = din("winn", (128, 18, 128))    # iw0hi*6 | iw0lo*6 | iw1*6

    outp = nc.dram_tensor("outp", [NCH * 128, 128], F32,
                          kind="ExternalOutput")
    outp_ap = outp.ap()

    with tile.TileContext(nc) as tc:
        with (
            tc.tile_pool(name="w", bufs=1) as wp,
            tc.tile_pool(name="big", bufs=1) as bigp,
            tc.tile_pool(name="work", bufs=6) as work,
            tc.tile_pool(name="psM", bufs=3, space=bass.MemorySpace.PSUM) as psM,
            tc.tile_pool(name="psA", bufs=2, space=bass.MemorySpace.PSUM) as psA,
            tc.tile_pool(name="psW", bufs=1, space=bass.MemorySpace.PSUM) as psW,
        ):
            wmsg = wp.tile([128, 3, 128], F32R, tag="wmsg")
            nap = [wp.tile([128, GW, NA], F32R, tag=f"nap{g}", name=f"nap{g}")
                   for g in range(NG)]
            bop = [wp.tile([32, GW, NA], F32R, tag=f"bop{g}", name=f"bop{g}")
                   for g in range(NG)]
            nact = wp.tile([128, NA], F32R, tag="nact")
            winn = wp.tile([128, 18, 128], F32R, tag="winn")
            # need-order, alternating issue queues
            nc.sync.dma_start(wmsg[:], wmsg_d[:])
            for g in range(NG):
                nc.scalar.dma_start(nap[g][:], nap_d[g][:])
                nc.sync.dma_start(bop[g][:], bop_d[g][:])
            nc.scalar.dma_start(nact[:], nact_d[:])
            nc.sync.dma_start(winn[:], winn_d[:])

            w0a = wmsg[:, 0, :]
            w0b = wmsg[0:32, 1, :]
            w1 = wmsg[:, 2, :]

            def iw0hi(d):
                return winn[:, d, :]

            def iw0lo(d):
                return winn[:, 6 + d, :]

            def iw1(d):
                return winn[:, 12 + d, :]

            # ---- PE clock-ramp burst (no data deps; runs during DMA wait) --
            if warmup:
                wz = wp.tile([128, 256], BF, tag="wz")
                nc.vector.memset(wz[:], 0.0)
                pw = psW.tile([128, 512], F32, tag="psW")
                for _ in range(warmup):
                    nc.tensor.matmul(pw[:, 0:256], wz[:, 0:128], wz[:, 0:256],
                                     start=True, stop=True)

            # ---- inner0 for the largest bucket: the actT (lo) matmul has
            # no msg dependency, so run it right after the pre-matmuls and
            # let the hi-matmuls accumulate once the partial sums exist ----
            Sg = [0] * D
            acc = 0
            for d in range(D):
                Sg[d] = acc
                acc += caps[d]
            dbig = int(np.argmax(caps))
            capb = caps[dbig]
            pibig = psW.tile([128, 512], F32, tag="psW")

            # ---- message MLP, two degree blocks per matmul ----
            # All first-layer matmuls are emitted before any second-layer
            # matmul: the PE executes its queue in order, so a late msg1
            # matmul must not block the next group's independent pre-matmuls.
            assert NA * GW <= 512
            m1 = bigp.tile([128, 6, NA], F32R, tag="m1")
            pms = []
            for g in range(NG):
                pm = psM.tile([128, 512], F32, tag="pm")
                pv = pm[:, 0:GW * NA]
                nc.tensor.matmul(pv, w0a,
                                 nap[g][:].rearrange("p a b -> p (a b)"),
                                 start=True, stop=False)
                nc.tensor.matmul(pv, w0b,
                                 bop[g][:].rearrange("p a b -> p (a b)"),
                                 start=False, stop=True)
                pms.append(pv)
            if capb <= 512:
                nc.tensor.matmul(pibig[:, 0:capb], iw0lo(dbig),
                                 nact[:, Sg[dbig]:Sg[dbig] + capb],
                                 start=True, stop=False)
            m0s = []
            for g in range(NG):
                pv = pms[g]
                e0 = work.tile([128, GW * NA], F32R, tag="e0")
                m0 = work.tile([128, GW * NA], F32R, tag="m0")
                nc.scalar.activation(e0[:], pv, AF.Exp)
                nc.vector._custom_dve(ELU_OP, out=m0[:], in0=pv, in1=e0[:],
                                      s0=1.0, s1=-1.0)
                m0s.append(m0)
            pm2s = []
            for g in range(NG):
                pm2 = psM.tile([128, 512], F32, tag="pm")
                pv2 = pm2[:, 0:GW * NA]
                nc.tensor.matmul(pv2, w1, m0s[g][:], start=True, stop=True)
                pm2s.append(pv2)
            for g in range(NG):
                e1 = work.tile([128, GW * NA], F32R, tag="e0")
                nc.scalar.activation(e1[:], pm2s[g], AF.Exp)
                nc.vector._custom_dve(
                    ELU_OP,
                    out=m1[:, GW * g:GW * g + GW, :].rearrange(
                        "p a b -> p (a b)"),
                    in0=pm2s[g], in1=e1[:], s0=1.0, s1=-1.0)

            # ---- d-sum: 3 independent pair adds (each ready right after
            # its group); inner0 matmuls accumulate the three partials ----
            sp = [bigp.tile([128, NA], F32R, tag=f"sp{g}", name=f"sp{g}")
                  for g in range(3)]
            for g in range(3):
                nc.vector.tensor_tensor(sp[g][:], m1[:, 2 * g, :],
                                        m1[:, 2 * g + 1, :], ALU.add)
            del GW, NG

            # ---- per-degree inner MLP, layer 0 (largest bucket first) ----
            h0 = bigp.tile([128, NA], F32R, tag="h0")
            S = Sg
            order = sorted(range(D), key=lambda d: -caps[d])
            for d in order:
                cap = caps[d]
                if cap == 0:
                    continue
                off = S[d]
                for s0 in range(0, cap, 512):
                    w = min(512, cap - s0)
                    early = d == dbig and capb <= 512
                    if early:
                        pi = pibig
                    else:
                        pi = psA.tile([128, 512], F32, tag="psA")
                        nc.tensor.matmul(pi[:, 0:w], iw0lo(d),
                                         nact[:, off + s0:off + s0 + w],
                                         start=True, stop=False)
                    for g in range(3):
                        nc.tensor.matmul(pi[:, 0:w], iw0hi(d),
                                         sp[g][:, off + s0:off + s0 + w],
                                         start=False, stop=(g == 2))
                    eh = work.tile([128, 512], F32R, tag="eh")
                    nc.scalar.activation(eh[:, 0:w], pi[:, 0:w], AF.Exp)
                    nc.vector._custom_dve(
                        ELU_OP, out=h0[:, off + s0:off + s0 + w],
                        in0=pi[:, 0:w], in1=eh[:, 0:w], s0=1.0, s1=-1.0)

            # ---- inner layer 1 -> single chunk-major output DMA ----
            obuf = bigp.tile([128, NCH, 128], F32, tag="obuf")
            nc.gpsimd.memset(obuf[:], 0.0)
            korder = sorted(range(NCH), key=lambda k: -chunks[k][2])
            for k in korder:
                d, s0, w = chunks[k]
                po = psA.tile([128, 512], F32, tag="psA")
                pov = po[0:w, 0:128]
                col = S[d] + s0
                nc.tensor.matmul(pov, h0[:, col:col + w], iw1(d),
                                 start=True, stop=True)
                eo = work.tile([128, 128], F32, tag="eo")
                nc.scalar.activation(eo[0:w, :], pov, AF.Exp)
                nc.vector._custom_dve(ELU_OP, out=obuf[0:w, k, :], in0=pov,
                                      in1=eo[0:w, :], s0=1.0, s1=-1.0)
            nc.sync.dma_start(
                outp_ap.rearrange("(k p) c -> p k c", p=128), obuf[:])

    nc.compile()
    return nc


_CACHE = {}


# --------------------------------------------------------------------------
# host side
# --------------------------------------------------------------------------

def _prep_core(atoms_c, bonds_c, edges_c, NA, caps):
    """Stage one core's arrays. Returns (dict name -> array, scatter info)."""
    af = atoms_c.reshape(NATOM, FA)
    bf = bonds_c.reshape(NATOM, D, FB)
    ef = edges_c.reshape(NATOM, D)
    deg = (ef != -1).sum(-1)

    act = np.nonzero(deg < D)[0]
    act = act[np.argsort(deg[act], kind="stable")]
    counts = np.bincount(deg[act], minlength=D)[:D]
    assert (counts <= np.asarray(caps)).all()

    S = np.concatenate([[0], np.cumsum(caps)])[:D]
    grid = np.full(NA, -1, np.int64)
    ofs = S.copy()
    for a in act:
        d = deg[a]
        grid[ofs[d]] = a
        ofs[d] += 1

    real = grid >= 0
    ga = grid[real]

    nbrT = np.zeros((128, D, NA), np.float32)
    e = ef[ga]
    mol = ga // M
    rcols = np.nonzero(real)[0]
    for d in range(D):
        has = e[:, d] >= 0
        nbrT[:, d, rcols[has]] = af[mol[has] * M + e[has, d]].T

    bo = np.zeros((32, D, NA), np.float32)
    bo[:, :, real] = bf[ga].transpose(2, 1, 0)
    nact = np.zeros((128, NA), np.float32)
    nact[:, real] = af[ga].T

    GW = 2 if 2 * NA <= 512 else 1
    m = dict(nact=nact)
    for g in range(D // GW):
        m[f"nap{g}"] = np.ascontiguousarray(nbrT[:, GW * g:GW * g + GW, :])
        m[f"bop{g}"] = np.ascontiguousarray(bo[:, GW * g:GW * g + GW, :])
    return m, ga, real


def _host_prep(atoms, bonds, edges):
    deg = (edges != -1).sum(-1).reshape(NCORES, NATOM)
    max_counts = np.zeros(D, np.int64)
    for c in range(NCORES):
        dc = deg[c]
        a = np.nonzero(dc < D)[0]
        cnt = np.bincount(dc[a], minlength=D)[:D]
        max_counts = np.maximum(max_counts, cnt)
    caps = [int(_roundup(x, 8)) if x > 0 else 0 for x in max_counts]
    NA = int(_roundup(max(sum(caps), 64), 16))
    caps[int(np.argmax(caps))] += NA - sum(caps)
    return NA, caps


def _pack_weights(msg_w0, msg_w1, inner_w0, inner_w1):
    wmsg = np.zeros((128, 3, 128), np.float32)
    wmsg[:, 0, :] = msg_w0[:128]
    wmsg[0:32, 1, :] = msg_w0[128:160]
    wmsg[:, 2, :] = msg_w1
    winn = np.zeros((128, 18, 128), np.float32)
    winn[:, 0:6, :] = inner_w0[:, :128, :].transpose(1, 0, 2)
    winn[:, 6:12, :] = inner_w0[:, 128:, :].transpose(1, 0, 2)
    winn[:, 12:18, :] = inner_w1.transpose(1, 0, 2)
    return wmsg, winn


def kernel(atoms, bonds, edges, msg_w0, msg_w1, inner_w0, inner_w1):
    atoms = np.asarray(atoms, np.float32)
    bonds = np.asarray(bonds, np.float32)
    edges = np.asarray(edges, np.int32)
    msg_w0 = np.asarray(msg_w0, np.float32)
    msg_w1 = np.asarray(msg_w1, np.float32)
    inner_w0 = np.asarray(inner_w0, np.float32)
    inner_w1 = np.asarray(inner_w1, np.float32)

    NA, caps = _host_prep(atoms, bonds, edges)

    key = (NA, tuple(caps))
    if key not in _CACHE:
        _CACHE[key] = build_program(NA, caps)
    nc = _CACHE[key]

    wmsg, winn = _pack_weights(msg_w0, msg_w1, inner_w0, inner_w1)

    in_maps = []
    scatter = []
    for c in range(NCORES):
        sl = slice(c * NMOL, (c + 1) * NMOL)
        m, ga, real = _prep_core(atoms[sl], bonds[sl], edges[sl], NA, caps)
        m["wmsg"] = wmsg
        m["winn"] = winn
        in_maps.append(m)
        scatter.append((ga, real))

    res = bass_utils.run_bass_kernel_spmd(
        nc, in_maps, core_ids=list(range(NCORES)))

    # unscatter: output rows are chunk-major (d, s0, w)
    chunks = _chunks(caps)
    S = np.concatenate([[0], np.cumsum(caps)])[:D]
    out = np.zeros((B * M, CONV), np.float32)
    for c in range(NCORES):
        ga, real = scatter[c]
        o = res.results[c]["outp"]
        full = np.zeros((NA, CONV), np.float32)
        for k, (d, s0, w) in enumerate(chunks):
            full[S[d] + s0:S[d] + s0 + w] = o[k * 128:k * 128 + w]
        out[c * NATOM + ga] = full[real]
    return out.reshape(B, M, CONV)
